# revision 1
# baseline (speedup 1.0000x reference)
"""Trainium2 Bass kernel for nn_CrossAttention (16x512x64x64, 8 heads x 64).

Math notes (exact algebraic restructuring of the reference):
  The reference tiles ky=[b,1,1,c] to k=[b,c,1,c] before conv1x1(to_k_w), so
  every input channel of that conv carries the same value ky[b,j].  Hence
    conv1x1(k, to_k_w)[b,o,0,j] = rowsum(to_k_w)[o] * ky[b,j]     (rank-1)
  and likewise for v with rowsum(to_v_w) and vy.  Propagating this:
    ksm[b,hd,j] = softmax_j(rs_k[hd] * ky[b,j])
    w[b,hd]     = sum_j ksm[b,hd,j] * vy[b,j]
    context[b,h,d,e] = w[b,h,d] * rs_v[h,e]                        (rank-1)
    out[b,he,n] = rs_v[he] * s[b,h,n],  s = sum_d softmax_d(q)[d,n] * w[h,d]
    final[b,o,n] = sum_h W2[o,h] * s[b,h,n] + out_b[o],
      with W2[o,h] = scale * sum_e out_w[o, h*64+e] * rs_v[h*64+e]
  followed by GroupNorm(1) over (C,H,W) per sample.

  The only large compute left is q = to_q_w @ x (2.1 GFLOP/sample), computed
  transposed (qT[n,he] = x[c,n]^T @ to_q_wT[c,he]) so the d-softmax is a
  free-dim reduction over 64-wide head chunks.

Sharding: data-parallel over batch, 2 samples per core, 8 cores, no
collectives.  Each core gets the full weights.
"""

import numpy as np

import concourse.bass as bass
import concourse.mybir as mybir
import concourse.tile as tile
from concourse import bacc
from concourse.bass import ts
from concourse.bass_utils import run_bass_kernel_spmd

B, C, N = 16, 512, 4096
DIMY = 768
HEADS, DHEAD = 8, 64
NCORES = 8
BPC = B // NCORES  # samples per core
SCALE = DHEAD ** -0.5
EPS = 1e-5
F32 = mybir.dt.float32
F32R = mybir.dt.float32r
BF16 = mybir.dt.bfloat16
AX = mybir.AxisListType.X
AF = mybir.ActivationFunctionType
OP = mybir.AluOpType


def build_nc(use_f32r=True):
    MDT = F32R if use_f32r else F32

    nc = bacc.Bacc()
    xd = nc.dram_tensor("x", [BPC, C, N], F32, kind="ExternalInput")
    yd = nc.dram_tensor("y", [BPC, DIMY], F32, kind="ExternalInput")
    kwd = nc.dram_tensor("k_w", [C, DIMY], F32, kind="ExternalInput")
    vwd = nc.dram_tensor("v_w", [C, DIMY], F32, kind="ExternalInput")
    qwd = nc.dram_tensor("to_q_w", [C, C], F32, kind="ExternalInput")
    tkd = nc.dram_tensor("to_k_w", [C, C], F32, kind="ExternalInput")
    tvd = nc.dram_tensor("to_v_w", [C, C], F32, kind="ExternalInput")
    owd = nc.dram_tensor("out_w", [C, C], F32, kind="ExternalInput")
    obd = nc.dram_tensor("out_b", [C], F32, kind="ExternalInput")
    gngd = nc.dram_tensor("gn_g", [C], F32, kind="ExternalInput")
    gnbd = nc.dram_tensor("gn_b", [C], F32, kind="ExternalInput")
    outd = nc.dram_tensor("out", [BPC, C, N], F32, kind="ExternalOutput")

    from contextlib import ExitStack

    with tile.TileContext(nc) as tc, ExitStack() as ctx:
        persist = ctx.enter_context(tc.tile_pool(name="persist", bufs=1))
        prep = ctx.enter_context(tc.tile_pool(name="prep", bufs=1))
        bcastp = ctx.enter_context(tc.tile_pool(name="bcast", bufs=5))
        ezp = ctx.enter_context(tc.tile_pool(name="ezp", bufs=2))
        eqp = ctx.enter_context(tc.tile_pool(name="eqp", bufs=3))
        workp = ctx.enter_context(tc.tile_pool(name="workp", bufs=3))
        xp = ctx.enter_context(tc.tile_pool(name="xp", bufs=10))
        sttp = ctx.enter_context(tc.tile_pool(name="sttp", bufs=18))
        stgp = ctx.enter_context(tc.tile_pool(name="stgp", bufs=6))
        tep = ctx.enter_context(tc.tile_pool(name="tep", bufs=6))
        smallp = ctx.enter_context(tc.tile_pool(name="smallp", bufs=6))
        rowp = ctx.enter_context(tc.tile_pool(name="rowp", bufs=2))
        statsp = ctx.enter_context(tc.tile_pool(name="statsp", bufs=2))
        ybcp = ctx.enter_context(tc.tile_pool(name="ybcp", bufs=1))
        psqp = ctx.enter_context(tc.tile_pool(name="psqp", bufs=3, space="PSUM"))
        psfp = ctx.enter_context(tc.tile_pool(name="psfp", bufs=3, space="PSUM"))
        psf2p = psfp
        psmp = ctx.enter_context(tc.tile_pool(name="psmp", bufs=2, space="PSUM"))

        def bcast_row(src_row_ap, n, tag, dt=F32):
            """Broadcast a [1, n] SBUF row to [128, n] via a K=1 PE matmul
            against a ones row (internal-DRAM scratch fails to load here)."""
            ps_b = psmp.tile([128, n], F32, tag="pm")
            nc.tensor.matmul(ps_b, lhsT=ones_row, rhs=src_row_ap, start=True, stop=True)
            b = bcastp.tile([128, n], dt, tag="bc" if n == C else "bc_" + tag)
            nc.scalar.copy(out=b, in_=ps_b)
            return b

        # ---------------- prep (sample independent) ----------------
        ident = persist.tile([128, 128], F32, tag="ident")
        from concourse.masks import make_identity

        make_identity(nc, ident)
        ones_col = persist.tile([128, 1], F32, tag="ones")
        nc.vector.memset(ones_col, 1.0)
        ones_row = persist.tile([1, 128], F32, tag="onesr")
        nc.vector.memset(ones_row, 1.0)
        zero_col = persist.tile([128, 1], F32, tag="zero")
        nc.vector.memset(zero_col, 0.0)
        nc.const_aps.aps[(F32, 0.0)] = zero_col[:, :]
        eps_col = persist.tile([128, 1], F32, tag="eps")
        nc.vector.memset(eps_col, EPS)
        nc.const_aps.aps[(F32, EPS)] = eps_col[:, :]

        # per-o columns [128, 4]: col i holds values for o in [i*128,(i+1)*128)
        outb_col = persist.tile([128, 4], F32, tag="outb")
        nc.sync.dma_start(out=outb_col, in_=obd.rearrange("(i p) -> p i", p=128))
        gng_col = persist.tile([128, 4], F32, tag="gng")
        nc.sync.dma_start(out=gng_col, in_=gngd.rearrange("(i p) -> p i", p=128))
        gnb_col = persist.tile([128, 4], F32, tag="gnb")
        nc.sync.dma_start(out=gnb_col, in_=gnbd.rearrange("(i p) -> p i", p=128))

        # to_q_w transposed -> qwT[:, ct, :] = to_q_w.T[ct*128:(ct+1)*128, :]
        tq_nat = prep.tile([128, 4, DIMY], F32, tag="wnat")
        nc.sync.dma_start(
            out=tq_nat[:, :, :C], in_=qwd.rearrange("(i p) c -> p i c", p=128)
        )
        qwT = persist.tile([128, 4, C], BF16, tag="qwT")
        for ct in range(4):
            for ot in range(4):
                pst = psmp.tile([128, 128], F32, tag="pm")
                nc.tensor.transpose(pst, tq_nat[:, ot, ts(ct, 128)], ident)
                nc.scalar.copy(out=qwT[:, ct, ts(ot, 128)], in_=pst)

        # row sums of to_k_w / to_v_w  -> [128, 4] columns
        rsk_col = persist.tile([128, 4], F32, tag="rsk")
        rsv_col = persist.tile([128, 4], F32, tag="rsv")
        for dram, col in ((tkd, rsk_col), (tvd, rsv_col)):
            nat = prep.tile([128, 4, DIMY], F32, tag="wnat")
            nc.sync.dma_start(
                out=nat[:, :, :C], in_=dram.rearrange("(i p) c -> p i c", p=128)
            )
            for ot in range(4):
                nc.vector.reduce_sum(
                    out=col[:, ot : ot + 1], in_=nat[:, ot, :C], axis=AX
                )

        # rs_v as a broadcast row, scaled by softmax scale (folded into W2)
        ps_row = psmp.tile([1, C], F32, tag="pm")
        for ot in range(4):
            nc.tensor.transpose(
                ps_row[:, ts(ot, 128)], rsv_col[:, ot : ot + 1], ident
            )
        rsv_row = rowp.tile([1, C], F32, tag="rsvrow")
        nc.scalar.mul(out=rsv_row, in_=ps_row, mul=SCALE)
        rsv_b = bcast_row(rsv_row, C, "rsv")

        # W2T[h, ot, :]: W2[o,h] = sum_e out_w[o, h*64+e] * rs_v[h*64+e] * scale
        ow_nat = prep.tile([128, 4, DIMY], F32, tag="wnat")
        nc.sync.dma_start(
            out=ow_nat[:, :, :C], in_=owd.rearrange("(i p) c -> p i c", p=128)
        )
        w2T = persist.tile([HEADS, 4, 128], MDT, tag="w2T")
        for ot in range(4):
            t_ = workp.tile([128, C], F32, tag="tmp")
            nc.vector.tensor_mul(t_, ow_nat[:, ot, :C], rsv_b)
            w2c = smallp.tile([128, HEADS], F32, tag="w2c")
            nc.vector.reduce_sum(
                out=w2c, in_=t_.rearrange("p (h d) -> p h d", d=DHEAD), axis=AX
            )
            psw = psmp.tile([HEADS, 128], F32, tag="pm")
            nc.tensor.transpose(psw, w2c, ident)
            nc.scalar.copy(out=w2T[:, ot, :], in_=psw)

        # ky / vy rows per sample: ky[b,o] = sum_d y[b,d] * k_w[o,d]
        kyvy = persist.tile([1, 2 * BPC, C], F32, tag="kyvy")  # [kv*BPC+s]
        for kv, dram in ((0, kwd), (1, vwd)):
            nat = prep.tile([128, 4, DIMY], F32, tag="kvnat")
            nc.sync.dma_start(out=nat, in_=dram.rearrange("(i p) d -> p i d", p=128))
            for s in range(BPC):
                y_b = ybcp.tile([128, DIMY], F32, tag="yb")
                nc.gpsimd.dma_start(out=y_b, in_=yd[s].partition_broadcast(128))
                col = smallp.tile([128, 4], F32, tag="kycol")
                for ot in range(4):
                    scr = ybcp.tile([128, DIMY], F32, tag="yscr")
                    nc.vector.tensor_mul(scr, nat[:, ot, :], y_b)
                    nc.vector.reduce_sum(
                        out=col[:, ot : ot + 1], in_=scr, axis=AX
                    )
                psr = psmp.tile([1, C], F32, tag="pm")
                for ot in range(4):
                    nc.tensor.transpose(
                        psr[:, ts(ot, 128)], col[:, ot : ot + 1], ident
                    )
                nc.scalar.copy(out=kyvy[:, kv * BPC + s, :], in_=psr)

        # ---------------- per-sample main ----------------
        for s in range(BPC):
            ky_b = bcast_row(kyvy[:, s, :], C, "ky")
            vy_b = bcast_row(kyvy[:, BPC + s, :], C, "vy")

            # k-softmax + weighting: w[hd] = sum_j softmax_j(rs_k[hd]*ky[j]) vy[j]
            den_k = smallp.tile([128, 4], F32, tag="denk")
            num_k = smallp.tile([128, 4], F32, tag="numk")
            for t in range(4):
                ez = ezp.tile([128, C], F32, tag="ez")
                nc.scalar.activation(
                    out=ez,
                    in_=ky_b,
                    func=AF.Exp,
                    scale=rsk_col[:, t : t + 1],
                )
                nc.vector.reduce_sum(
                    out=den_k[:, t : t + 1], in_=ez, axis=AX
                )
                scr = workp.tile([128, C], F32, tag="tmp")
                nc.vector.tensor_mul(scr, ez, vy_b)
                nc.vector.reduce_sum(
                    out=num_k[:, t : t + 1], in_=scr, axis=AX
                )
            denr_k = smallp.tile([128, 4], F32, tag="denrk")
            nc.vector.reciprocal(denr_k, den_k)
            w_col = smallp.tile([128, 4], F32, tag="wcol")
            nc.vector.tensor_mul(w_col, num_k, denr_k)
            ps_w = psmp.tile([1, C], F32, tag="pm")
            for t in range(4):
                nc.tensor.transpose(ps_w[:, ts(t, 128)], w_col[:, t : t + 1], ident)
            w_row = rowp.tile([1, C], F32, tag="wrow")
            nc.scalar.copy(out=w_row, in_=ps_w)
            w_b = bcast_row(w_row, C, "w", dt=BF16)

            stats = statsp.tile([128, 4, 8, 6], F32, tag="stats")
            stt_tiles = []
            for g in range(8):  # n-groups of 512
                xcs = []
                for ct in range(4):
                    xc = xp.tile([128, 512], BF16, tag="xc")
                    nc.gpsimd.dma_start(
                        out=xc,
                        in_=xd[s, ts(ct, 128), ts(g, 512)],
                    )
                    xcs.append(xc)
                ps_stt = psmp.tile([HEADS, 512], F32, tag="pm")
                for j in range(4):  # n-tiles of 128 within the group
                    psq = psqp.tile([128, 512], F32, tag="psq")
                    for ct in range(4):
                        nc.tensor.matmul(
                            psq,
                            lhsT=xcs[ct][:, ts(j, 128)],
                            rhs=qwT[:, ct, :],
                            start=(ct == 0),
                            stop=(ct == 3),
                        )
                    te = tep.tile([128, 2, 512], BF16, tag="te")
                    nc.scalar.activation(out=te[:, 1, :], in_=psq, func=AF.Exp)
                    nc.gpsimd.tensor_mul(te[:, 0, :], te[:, 1, :], w_b)
                    sn2 = smallp.tile([128, 2, HEADS], F32, tag="sn2")
                    nc.vector.reduce_sum(
                        out=sn2,
                        in_=te.rearrange("p t (h d) -> p t h d", d=DHEAD),
                        axis=AX,
                    )
                    s_denr = smallp.tile([128, HEADS], F32, tag="sdenr")
                    nc.vector.reciprocal(s_denr, sn2[:, 1, :])
                    s_t = smallp.tile([128, HEADS], F32, tag="stile")
                    nc.vector.tensor_mul(s_t, sn2[:, 0, :], s_denr)
                    nc.tensor.transpose(ps_stt[:, ts(j, 128)], s_t, ident)
                stt = sttp.tile([HEADS, 512], MDT, tag="stt")
                nc.scalar.copy(out=stt, in_=ps_stt)
                stt_tiles.append(stt)
                for ot in range(4):
                    psf = psfp.tile([128, 512], F32, tag="psf")
                    nc.tensor.matmul(
                        psf,
                        lhsT=w2T[:, ot, :],
                        rhs=stt,
                        start=True,
                        stop=True,
                    )
                    nc.vector.bn_stats(out=stats[:, ot, g, :], in_=psf)

            # ---- GroupNorm(1) stats over the whole sample ----
            mvacc = smallp.tile([128, 2, 4], F32, tag="mvacc")
            for ot in range(4):
                mv = smallp.tile([128, 2], F32, tag="mv")
                nc.vector.bn_aggr(out=mv, in_=stats[:, ot, :, :])
                m_ = mvacc[:, 0, ot : ot + 1]
                nc.vector.tensor_add(m_, mv[:, 0:1], outb_col[:, ot : ot + 1])
                msq = smallp.tile([128, 1], F32, tag="msq")
                nc.vector.tensor_mul(msq, m_, m_)
                nc.vector.tensor_add(mvacc[:, 1, ot : ot + 1], mv[:, 1:2], msq)
            mv_tot = smallp.tile([128, 2], F32, tag="mvtot")
            nc.vector.reduce_sum(out=mv_tot, in_=mvacc, axis=AX)
            ps_tot = psmp.tile([1, 2], F32, tag="pm")
            nc.tensor.matmul(ps_tot, lhsT=ones_col, rhs=mv_tot, start=True, stop=True)
            tt = rowp.tile([1, 4], F32, tag="tt")
            nc.scalar.mul(out=tt[:, 0:2], in_=ps_tot, mul=1.0 / C)
            nc.vector.tensor_mul(tt[:, 2:3], tt[:, 0:1], tt[:, 0:1])  # mu^2
            nc.vector.tensor_sub(tt[:, 3:4], tt[:, 1:2], tt[:, 2:3])  # var
            sd = rowp.tile([1, 1], F32, tag="sd")
            nc.scalar.activation(out=sd, in_=tt[:, 3:4], func=AF.Sqrt, bias=EPS)
            rstd = rowp.tile([1, 1], F32, tag="rstd")
            nc.vector.reciprocal(rstd, sd)
            murow = rowp.tile([1, 2], F32, tag="mur")
            nc.vector.tensor_copy(murow[:, 0:1], tt[:, 0:1])
            nc.vector.tensor_copy(murow[:, 1:2], rstd)
            ms_b = bcast_row(murow, 2, "ms")

            # A = gn_g * rstd ; B = A*(out_b - mu) + gn_b ; out = A*mm + B
            a_col = smallp.tile([128, 4], F32, tag="acol")
            nc.vector.tensor_scalar_mul(a_col, gng_col, ms_b[:, 1:2])
            t1 = smallp.tile([128, 4], F32, tag="t1")
            nc.vector.tensor_scalar(
                out=t1, in0=outb_col, scalar1=ms_b[:, 0:1], scalar2=None,
                op0=OP.subtract,
            )
            t2 = smallp.tile([128, 4], F32, tag="t2")
            nc.vector.tensor_mul(t2, a_col, t1)
            b_col = smallp.tile([128, 4], F32, tag="bcol")
            nc.vector.tensor_add(b_col, t2, gnb_col)

            # rows: A and B2 as [1, 512] rows, A broadcast to 8 partitions
            ps_a = psmp.tile([1, C], F32, tag="pm")
            for ot in range(4):
                nc.tensor.transpose(
                    ps_a[:, ts(ot, 128)], a_col[:, ot : ot + 1], ident
                )
            a_row = rowp.tile([1, C], F32, tag="arow")
            nc.scalar.copy(out=a_row, in_=ps_a)
            ps_a8 = psmp.tile([HEADS, C], F32, tag="pm")
            nc.tensor.matmul(
                ps_a8,
                lhsT=ones_row[:, 0:HEADS],
                rhs=a_row,
                start=True,
                stop=True,
            )
            a8_sb = rowp.tile([HEADS, C], F32, tag="a8")
            nc.scalar.copy(out=a8_sb, in_=ps_a8)
            # w2s = W2T * A(o); B2(o) is added as bias in the staging copy
            w2s = rowp.tile([HEADS, 4, 128], MDT, tag="w2s")
            nc.vector.tensor_mul(
                w2s,
                w2T,
                a8_sb.rearrange("p (i f) -> p i f", i=4),
            )
            for g in range(8):
                for ot in range(4):
                    psf2 = psf2p.tile([128, 512], F32, tag="psf")
                    nc.tensor.matmul(
                        psf2,
                        lhsT=w2s[:, ot, :],
                        rhs=stt_tiles[g],
                        start=True,
                        stop=True,
                    )
                    stg = stgp.tile([128, 512], F32, tag="stg")
                    nc.scalar.activation(
                        out=stg,
                        in_=psf2,
                        func=AF.Identity,
                        bias=b_col[:, ot : ot + 1],
                    )
                    nc.sync.dma_start(
                        out=outd[s, ts(ot, 128), ts(g, 512)], in_=stg
                    )

    nc.finalize()
    return nc


_NC_CACHE = {}


def _get_nc(use_f32r=True):
    if use_f32r not in _NC_CACHE:
        _NC_CACHE[use_f32r] = build_nc(use_f32r)
    return _NC_CACHE[use_f32r]


def make_in_maps(inputs):
    x = np.ascontiguousarray(inputs["x"], dtype=np.float32).reshape(B, C, N)
    y = np.ascontiguousarray(inputs["y"], dtype=np.float32).reshape(B, DIMY)
    shared = {
        k: np.ascontiguousarray(inputs[k], dtype=np.float32)
        for k in (
            "k_w", "v_w", "to_q_w", "to_k_w", "to_v_w", "out_w",
            "out_b", "gn_g", "gn_b",
        )
    }
    in_maps = []
    for core in range(NCORES):
        s0 = core * BPC
        m = {"x": x[s0 : s0 + BPC], "y": y[s0 : s0 + BPC]}
        m.update(shared)
        in_maps.append(m)
    return in_maps


def kernel(**inputs):
    nc = _get_nc(use_f32r=True)
    res = run_bass_kernel_spmd(nc, make_in_maps(inputs), list(range(NCORES)))
    out = np.concatenate([r["out"] for r in res.results], axis=0)
    return out.reshape(B, C, 64, 64)


if __name__ == "__main__":
    rng = np.random.default_rng(0)
    inputs = {
        "x": rng.standard_normal((B, C, 64, 64), dtype=np.float32),
        "y": rng.standard_normal((B, 1, 1, DIMY), dtype=np.float32),
        "k_w": rng.standard_normal((C, DIMY), dtype=np.float32) * 0.02,
        "v_w": rng.standard_normal((C, DIMY), dtype=np.float32) * 0.02,
        "to_q_w": rng.standard_normal((C, C), dtype=np.float32) * 0.02,
        "to_k_w": rng.standard_normal((C, C), dtype=np.float32) * 0.02,
        "to_v_w": rng.standard_normal((C, C), dtype=np.float32) * 0.02,
        "out_w": rng.standard_normal((C, C), dtype=np.float32) * 0.02,
        "out_b": np.zeros(C, np.float32),
        "gn_g": np.ones(C, np.float32),
        "gn_b": np.zeros(C, np.float32),
    }
    out = kernel(**inputs)
    print("kernel ran, out shape", out.shape, "std", out.std())



# revision 11
# speedup vs baseline: 1.2983x; 1.2983x over previous
"""Trainium2 Bass kernel for nn_CrossAttention (16x512x64x64, 8 heads x 64).

Math notes (exact algebraic restructuring of the reference):
  The reference tiles ky=[b,1,1,c] to k=[b,c,1,c] before conv1x1(to_k_w), so
  every input channel of that conv carries the same value ky[b,j].  Hence
    conv1x1(k, to_k_w)[b,o,0,j] = rowsum(to_k_w)[o] * ky[b,j]     (rank-1)
  and likewise for v with rowsum(to_v_w) and vy.  Propagating this:
    ksm[b,hd,j] = softmax_j(rs_k[hd] * ky[b,j])
    w[b,hd]     = sum_j ksm[b,hd,j] * vy[b,j]
    s[b,h,n]    = num/den,  num = sum_d w[hd] e^{q[hd,n]}, den = sum_d e^{q[hd,n]}
    mm[b,o,n]   = sum_h W2[o,h] * s[b,h,n] + out_b[o],
      with W2[o,h] = scale * sum_e out_w[o, h*64+e] * rs_v[h*64+e]
  followed by GroupNorm(1) over (C,H,W) per sample:
    out = A[o]*mmW2[o,n] + B[o],  A = gn_g*rstd, B = A*(out_b-mu)+gn_b
  GN stats come from the 9x9 Gram matrix of [s; 1] over n:
    sum mm   = sum_ab Cm[a,b] S2[a,b],   sum mm^2 = sum_ab Gm[a,b] S2[a,b]
  where S2 = [s;1][s;1]^T (accumulated on PE), Cm/Gm folded on host from
  W2 / out_b.

Device layout: q kept transposed [he, n] so the d-softmax reductions are
small PE matmuls (block-diagonal masks carrying w), not DVE reductions.
The only large compute is the q GEMM (to_q_w @ x, bf16, 2.1 GFLOP/sample).

Sharding: data-parallel over batch, 2 samples per core, 8 cores, no
collectives.  Weight folding (transposes, rowsums, W2, Gm/Cm) is done on
host; x is staged to bf16 on host.
"""

import numpy as np
import ml_dtypes

import concourse.bass as bass
import concourse.mybir as mybir
import concourse.tile as tile
from concourse import bacc
from concourse.bass import ts
from concourse.bass_utils import run_bass_kernel_spmd

B, C, N = 16, 512, 4096
DIMY = 768
HEADS, DHEAD = 8, 64
NCORES = 8
BPC = B // NCORES  # samples per core
SCALE = DHEAD ** -0.5
EPS = 1e-5
F32 = mybir.dt.float32
BF16 = mybir.dt.bfloat16
AX = mybir.AxisListType.X
AF = mybir.ActivationFunctionType
OP = mybir.AluOpType
NG = 8          # n-groups of 512 per sample
GSZ = N // NG   # 512
CN = C * N

BF = ml_dtypes.bfloat16


def build_nc():
    nc = bacc.Bacc()
    xd = nc.dram_tensor("x", [BPC, 128, 4, N], BF16, kind="ExternalInput")
    qwTd = nc.dram_tensor("qwT", [128, 4, C], BF16, kind="ExternalInput")
    kvTd = nc.dram_tensor("kvT", [128, 6, 2 * C], BF16, kind="ExternalInput")
    yTd = nc.dram_tensor("yT", [128, 6, BPC], BF16, kind="ExternalInput")
    rskbd = nc.dram_tensor("rskb", [128, C], F32, kind="ExternalInput")
    omaskd = nc.dram_tensor("omask", [128, 4, HEADS], BF16, kind="ExternalInput")
    w2Td = nc.dram_tensor("w2T", [HEADS, 4, 128], BF16, kind="ExternalInput")
    gcmd = nc.dram_tensor("gcm", [9, 2, 9], F32, kind="ExternalInput")
    colsd = nc.dram_tensor("cols", [128, 12], F32, kind="ExternalInput")
    outd = nc.dram_tensor("out", [BPC, 4, 128, N], BF16, kind="ExternalOutput")

    from contextlib import ExitStack

    with tile.TileContext(nc) as tc, ExitStack() as ctx:
        persist = ctx.enter_context(tc.tile_pool(name="persist", bufs=1))
        xp = ctx.enter_context(tc.tile_pool(name="xp", bufs=2))
        ep = ctx.enter_context(tc.tile_pool(name="ep", bufs=3))
        stgp = ctx.enter_context(tc.tile_pool(name="stgp", bufs=4))
        samp = ctx.enter_context(tc.tile_pool(name="samp", bufs=2))
        gp = ctx.enter_context(tc.tile_pool(name="gp", bufs=3))
        tiny = ctx.enter_context(tc.tile_pool(name="tiny", bufs=4))
        psqp = ctx.enter_context(tc.tile_pool(name="psqp", bufs=2, space="PSUM"))
        psndp = ctx.enter_context(tc.tile_pool(name="psndp", bufs=2, space="PSUM"))
        psfp = ctx.enter_context(tc.tile_pool(name="psfp", bufs=2, space="PSUM"))
        psgp = ctx.enter_context(tc.tile_pool(name="psgp", bufs=1, space="PSUM"))
        pssm = ctx.enter_context(tc.tile_pool(name="pssm", bufs=1, space="PSUM"))

        # ---------------- prep: weights + constants ----------------
        qwT = persist.tile([128, 4, C], BF16, tag="qwT")
        nc.sync.dma_start(out=qwT, in_=qwTd[:, :, :])
        kvT = persist.tile([128, 6, 2 * C], BF16, tag="kvT")
        nc.sync.dma_start(out=kvT, in_=kvTd[:, :, :])
        yT = persist.tile([128, 6, BPC], BF16, tag="yT")
        nc.sync.dma_start(out=yT, in_=yTd[:, :, :])
        rskb = persist.tile([128, C], F32, tag="rskb")
        nc.sync.dma_start(out=rskb, in_=rskbd[:, :])
        omask = persist.tile([128, 4, HEADS], BF16, tag="omask")
        nc.sync.dma_start(out=omask, in_=omaskd[:, :, :])
        w2T = persist.tile([HEADS, 4, 128], BF16, tag="w2T")
        nc.sync.dma_start(out=w2T, in_=w2Td[:, :, :])
        gcm = persist.tile([9, 2, 9], F32, tag="gcm")
        nc.sync.dma_start(out=gcm, in_=gcmd[:, :, :])
        gcols = persist.tile([128, 12], F32, tag="gcols")
        nc.sync.dma_start(out=gcols, in_=colsd[:, :])

        ident = persist.tile([128, 128], F32, tag="ident")
        from concourse.masks import make_identity

        make_identity(nc, ident)
        identb = persist.tile([128, 128], BF16, tag="identb")
        make_identity(nc, identb)
        ones_row = persist.tile([1, 128], F32, tag="onesr")
        nc.vector.memset(ones_row, 1.0)
        ones9 = persist.tile([9, 1], F32, tag="ones9")
        nc.vector.memset(ones9, 1.0)
        zero_col = persist.tile([128, 1], F32, tag="zero")
        nc.vector.memset(zero_col, 0.0)
        nc.const_aps.aps[(F32, 0.0)] = zero_col[:, :]
        eps_col = persist.tile([128, 1], F32, tag="eps")
        nc.vector.memset(eps_col, EPS)
        nc.const_aps.aps[(F32, EPS)] = eps_col[:, :]
        # gram staging: [128 n, 4 j, 9]; col 8 of each j-block stays 1.0
        gstage = persist.tile([128, 4, 9], BF16, tag="gstage")
        nc.vector.memset(gstage[:, :, 8:9], 1.0)
        # s values for both samples: [8 h, s, n]
        s_all = persist.tile([HEADS, BPC, N], BF16, tag="s_all")

        # x for sample 0 (scalar-engine queue), sample 1 (sync queue)
        xts = []
        for s in range(BPC):
            xt = xp.tile([128, 4, N], BF16, tag="xt")
            eng = nc.scalar if s == 0 else nc.sync
            eng.dma_start(out=xt, in_=xd[s])
            xts.append(xt)

        # ---------------- ky/vy for both samples ----------------
        # ky[s, o] = sum_d y[s, d] k_w[o, d]; vy likewise (both via PE)
        krows = tiny.tile([BPC, 2, C], F32, tag="krows")
        for kv in range(2):
            ps_ky = pssm.tile([BPC, C], F32, tag="sm")
            for dt_ in range(6):
                nc.tensor.matmul(
                    ps_ky, lhsT=yT[:, dt_, :], rhs=kvT[:, dt_, kv * C : (kv + 1) * C],
                    start=(dt_ == 0), stop=(dt_ == 5),
                )
            nc.vector.tensor_copy(out=krows[:, kv, :], in_=ps_ky)
        # transpose to columns: kv_cols[p, 4*ot + 2*kv + s]
        ps_kc = pssm.tile([128, 16], F32, tag="sm")
        for ot in range(4):
            for kv in range(2):
                nc.tensor.transpose(
                    ps_kc[:, 4 * ot + 2 * kv : 4 * ot + 2 * kv + 2],
                    krows[:, kv, ts(ot, 128)],
                    ident[0:BPC, 0:BPC],
                )
        kv_cols = persist.tile([128, 16], F32, tag="kvcols")
        nc.vector.tensor_copy(out=kv_cols, in_=ps_kc)

        per_sample = []  # (numq, ab) tiles
        for s in range(BPC):
            # ---- k-softmax -> w ----
            # E_T[j, hd] = exp(ky[j] * rs_k[hd])
            et = ep.tile([128, 4, C], BF16, tag="eq")
            for jt in range(4):
                nc.scalar.activation(
                    out=et[:, jt, :], in_=rskb, func=AF.Exp,
                    scale=kv_cols[:, 4 * jt + s : 4 * jt + s + 1],
                )
            # masks: col 0 = vy (num), col 32 = 1 (den at psum partition 32)
            kvm = tiny.tile([128, 4, 33], BF16, tag="kvm")
            nc.vector.memset(kvm, 0.0)
            nc.vector.tensor_copy(
                out=kvm[:, :, 0:1],
                in_=kv_cols.rearrange("p (a r) -> p a r", r=4)[:, :, 2 + s : 3 + s],
            )
            nc.vector.memset(kvm[:, :, 32:33], 1.0)
            ps_w = pssm.tile([33, C], F32, tag="sm")
            for jt in range(4):
                nc.tensor.matmul(
                    ps_w, lhsT=kvm[:, jt, :], rhs=et[:, jt, :],
                    start=(jt == 0), stop=(jt == 3),
                )
            rdw = tiny.tile([1, C], F32, tag="rdw")
            nc.vector.reciprocal(rdw, ps_w[32:33, :])
            w_row = tiny.tile([1, C], F32, tag="wrow")
            nc.vector.tensor_mul(w_row, ps_w[0:1, :], rdw)
            ps_wc = pssm.tile([128, 4], F32, tag="sm")
            for ht in range(4):
                nc.tensor.transpose(
                    ps_wc[:, ht : ht + 1], w_row[:, ts(ht, 128)], ident[0:1, 0:1]
                )
            w_col = tiny.tile([128, 4], F32, tag="wcol")
            nc.vector.tensor_copy(out=w_col, in_=ps_wc)
            # numq masks: cols 0:8 = omask * w (num), cols 32:40 = omask (den)
            # (den lands at psum partitions 32-39 so the reciprocal's read
            #  stays 32-aligned)
            numq = samp.tile([128, 4, 48], BF16, tag="numq")
            nc.vector.memset(numq, 0.0)
            for ht in range(4):
                nc.vector.tensor_scalar(
                    out=numq[:, ht, 0:HEADS], in0=omask[:, ht, :],
                    scalar1=w_col[:, ht : ht + 1], scalar2=None, op0=OP.mult,
                )
            nc.vector.tensor_copy(out=numq[:, :, 32:40], in_=omask)

            # ---- pass 1: q GEMM + softmax-reduce + Gram ----
            psg = psgp.tile([9, 4, 9], F32, tag="gram")
            for g in range(NG):
                eq = ep.tile([128, 4, C], BF16, tag="eq")
                for ht in range(4):
                    psq = psqp.tile([128, GSZ], F32, tag="psq")
                    for ct in range(4):
                        nc.tensor.matmul(
                            psq,
                            lhsT=qwT[:, ct, ts(ht, 128)],
                            rhs=xts[s][:, ct, ts(g, GSZ)],
                            start=(ct == 0), stop=(ct == 3),
                        )
                    nc.scalar.activation(out=eq[:, ht, :], in_=psq, func=AF.Exp)
                psnd = psndp.tile([48, GSZ], F32, tag="nd")
                for ht in range(4):
                    nc.tensor.matmul(
                        psnd, lhsT=numq[:, ht, :], rhs=eq[:, ht, :],
                        start=(ht == 0), stop=(ht == 3),
                    )
                rden = gp.tile([HEADS, GSZ], F32, tag="rden")
                nc.vector.reciprocal(rden, psnd[32:40, :])
                nc.vector.tensor_mul(
                    s_all[:, s, ts(g, GSZ)], psnd[0:HEADS, :], rden
                )
                ps_sT = pssm.tile([128, 4 * HEADS], BF16, tag="sm")
                for j in range(4):
                    nc.tensor.transpose(
                        ps_sT[:, 8 * j : 8 * j + 8],
                        s_all[:, s, g * GSZ + 128 * j : g * GSZ + 128 * (j + 1)],
                        identb[0:HEADS, 0:HEADS],
                    )
                nc.vector.tensor_copy(
                    out=gstage[:, :, 0:HEADS],
                    in_=ps_sT.rearrange("p (j h) -> p j h", h=HEADS),
                )
                for j in range(4):
                    nc.tensor.matmul(
                        psg[:, j, :], lhsT=gstage[:, j, :], rhs=gstage[:, j, :],
                        start=(g == 0), stop=(g == NG - 1), skip_group_check=True,
                    )

            # ---- GN stats from Gram ----
            gsb = tiny.tile([9, 4, 9], F32, tag="gsb")
            nc.vector.tensor_copy(out=gsb, in_=psg)
            s2 = tiny.tile([9, 9], F32, tag="s2")
            nc.vector.reduce_sum(
                out=s2, in_=gsb.rearrange("p j b -> p b j"), axis=AX
            )
            work = tiny.tile([9, 2, 9], F32, tag="work")
            nc.vector.tensor_mul(work[:, 0, :], gcm[:, 0, :], s2)
            nc.vector.tensor_mul(work[:, 1, :], gcm[:, 1, :], s2)
            wred = tiny.tile([9, 2], F32, tag="wred")
            nc.vector.reduce_sum(out=wred, in_=work, axis=AX)
            ps_s = pssm.tile([1, 2], F32, tag="sm")
            nc.tensor.matmul(ps_s, lhsT=ones9, rhs=wred, start=True, stop=True)
            msc = tiny.tile([1, 4], F32, tag="msc")
            nc.scalar.mul(out=msc[:, 0:2], in_=ps_s, mul=1.0 / CN)
            nc.vector.tensor_mul(msc[:, 2:3], msc[:, 0:1], msc[:, 0:1])
            nc.vector.tensor_sub(msc[:, 3:4], msc[:, 1:2], msc[:, 2:3])
            lnv = tiny.tile([1, 2], F32, tag="lnv")
            nc.scalar.activation(out=lnv[:, 0:1], in_=msc[:, 3:4], func=AF.Ln, bias=EPS)
            nc.scalar.activation(out=lnv[:, 1:2], in_=lnv[:, 0:1], func=AF.Exp, scale=-0.5)
            murow = tiny.tile([1, 2], F32, tag="murow")
            nc.vector.tensor_copy(out=murow[:, 0:1], in_=msc[:, 0:1])
            nc.vector.tensor_copy(out=murow[:, 1:2], in_=lnv[:, 1:2])
            ps_b = pssm.tile([128, 2], F32, tag="sm")
            nc.tensor.matmul(ps_b, lhsT=ones_row, rhs=murow, start=True, stop=True)
            msb = tiny.tile([128, 2], F32, tag="msb")
            nc.vector.tensor_copy(out=msb, in_=ps_b)
            # A = gn_g * rstd ; B = A*(out_b - mu) + gn_b
            ab = samp.tile([128, 2, 4], F32, tag="ab")
            nc.vector.tensor_scalar(
                out=ab[:, 0, :], in0=gcols[:, 0:4],
                scalar1=msb[:, 1:2], scalar2=None, op0=OP.mult,
            )
            t1 = tiny.tile([128, 2, 4], F32, tag="t1")
            nc.vector.tensor_scalar(
                out=t1[:, 0, :], in0=gcols[:, 8:12],
                scalar1=msb[:, 0:1], scalar2=None, op0=OP.subtract,
            )
            nc.vector.tensor_mul(t1[:, 1, :], ab[:, 0, :], t1[:, 0, :])
            nc.vector.tensor_add(ab[:, 1, :], t1[:, 1, :], gcols[:, 4:8])
            per_sample.append((numq, ab))

        # ---------------- pass 2: mm = W2 @ s, affine, store ----------------
        for s in range(BPC):
            ab = per_sample[s][1]
            for g in range(NG):
                for ot in range(4):
                    psf = psfp.tile([128, GSZ], F32, tag="psf")
                    nc.tensor.matmul(
                        psf, lhsT=w2T[:, ot, :], rhs=s_all[:, s, ts(g, GSZ)],
                        start=True, stop=True,
                    )
                    stg = stgp.tile([128, GSZ], BF16, tag="stg")
                    nc.vector.tensor_scalar(
                        out=stg, in0=psf,
                        scalar1=ab[:, 0, ot : ot + 1], scalar2=ab[:, 1, ot : ot + 1],
                        op0=OP.mult, op1=OP.add,
                    )
                    nc.gpsimd.dma_start(out=outd[s, ot, :, ts(g, GSZ)], in_=stg)

    nc.finalize()
    return nc


_NC_CACHE = {}


def _get_nc():
    if "nc" not in _NC_CACHE:
        _NC_CACHE["nc"] = build_nc()
    return _NC_CACHE["nc"]


def _fold_host(inputs):
    """Host-side weight folding + staging (shared across cores)."""
    k_w = np.asarray(inputs["k_w"], np.float32)
    v_w = np.asarray(inputs["v_w"], np.float32)
    to_q_w = np.asarray(inputs["to_q_w"], np.float32)
    to_k_w = np.asarray(inputs["to_k_w"], np.float32)
    to_v_w = np.asarray(inputs["to_v_w"], np.float32)
    out_w = np.asarray(inputs["out_w"], np.float32)
    out_b = np.asarray(inputs["out_b"], np.float32)
    gn_g = np.asarray(inputs["gn_g"], np.float32)
    gn_b = np.asarray(inputs["gn_b"], np.float32)

    qwT = np.ascontiguousarray(
        to_q_w.T.reshape(4, 128, C).transpose(1, 0, 2)
    ).astype(BF)  # [128, ct, he]
    kT = k_w.T.reshape(6, 128, C).transpose(1, 0, 2)  # [128, dt, o]
    vT = v_w.T.reshape(6, 128, C).transpose(1, 0, 2)
    kvT = np.ascontiguousarray(np.concatenate([kT, vT], axis=2)).astype(BF)

    rs_k = to_k_w.sum(axis=1)  # [C]
    rs_v = to_v_w.sum(axis=1)
    rskb = np.ascontiguousarray(np.broadcast_to(rs_k[None, :], (128, C))).astype(
        np.float32
    )

    # W2[o, h] = scale * sum_e out_w[o, h*64+e] * rs_v[h*64+e]
    W2 = SCALE * np.einsum(
        "ohe,he->oh", out_w.reshape(C, HEADS, DHEAD), rs_v.reshape(HEADS, DHEAD)
    )  # [C, HEADS]
    w2T = np.ascontiguousarray(
        W2.reshape(4, 128, HEADS).transpose(2, 0, 1)
    ).astype(BF)  # [h, ot, p]

    # Gm/Cm: sum mm^q = sum_ab M[a,b] S2[a,b], S2 = [s;1][s;1]^T over n
    G = W2.T @ W2  # [8, 8]
    colsumW2 = W2.sum(axis=0)  # [8]
    bW2 = out_b @ W2  # [8]
    Gm = np.zeros((9, 9), np.float32)
    Gm[:8, :8] = G
    Gm[8, :8] = bW2
    Gm[:8, 8] = bW2
    Gm[8, 8] = float((out_b ** 2).sum())
    Cm = np.zeros((9, 9), np.float32)
    Cm[8, :8] = colsumW2 / 2.0
    Cm[:8, 8] = colsumW2 / 2.0
    Cm[8, 8] = float(out_b.sum())
    gcm = np.ascontiguousarray(
        np.stack([Cm, Gm], axis=1)
    ).astype(np.float32)  # [9, 2, 9]

    omask = np.zeros((128, 4, HEADS), np.float32)
    for ht in range(4):
        for p in range(128):
            omask[p, ht, 2 * ht + p // 64] = 1.0
    omask = omask.astype(BF)

    cols = np.ascontiguousarray(
        np.stack(
            [*gn_g.reshape(4, 128), *gn_b.reshape(4, 128), *out_b.reshape(4, 128)],
            axis=1,
        )
    ).astype(np.float32)  # [128, 12]
    return dict(qwT=qwT, kvT=kvT, rskb=rskb, omask=omask, w2T=w2T, gcm=gcm, cols=cols)


def make_in_maps(inputs):
    x = np.asarray(inputs["x"], np.float32).reshape(B, 4, 128, N)
    x = np.ascontiguousarray(x).astype(BF)
    y = np.asarray(inputs["y"], np.float32).reshape(B, DIMY)
    shared = _fold_host(inputs)
    in_maps = []
    for core in range(NCORES):
        s0 = core * BPC
        yc = y[s0 : s0 + BPC]  # [BPC, DIMY]
        yT = np.ascontiguousarray(
            yc.T.reshape(6, 128, BPC).transpose(1, 0, 2)
        ).astype(BF)
        m = {"x": x[s0 : s0 + BPC].transpose(0, 2, 1, 3).copy(), "yT": yT}
        m.update(shared)
        in_maps.append(m)
    return in_maps


def kernel(**inputs):
    nc = _get_nc()
    res = run_bass_kernel_spmd(nc, make_in_maps(inputs), list(range(NCORES)))
    out = np.concatenate([r["out"] for r in res.results], axis=0)  # [B, 4, 128, N] bf16
    return out.astype(np.float32).reshape(B, C, 64, 64)


if __name__ == "__main__":
    rng = np.random.default_rng(0)
    inputs = {
        "x": rng.standard_normal((B, C, 64, 64), dtype=np.float32),
        "y": rng.standard_normal((B, 1, 1, DIMY), dtype=np.float32),
        "k_w": rng.standard_normal((C, DIMY), dtype=np.float32) * 0.02,
        "v_w": rng.standard_normal((C, DIMY), dtype=np.float32) * 0.02,
        "to_q_w": rng.standard_normal((C, C), dtype=np.float32) * 0.02,
        "to_k_w": rng.standard_normal((C, C), dtype=np.float32) * 0.02,
        "to_v_w": rng.standard_normal((C, C), dtype=np.float32) * 0.02,
        "out_w": rng.standard_normal((C, C), dtype=np.float32) * 0.02,
        "out_b": np.zeros(C, np.float32),
        "gn_g": np.ones(C, np.float32),
        "gn_b": np.zeros(C, np.float32),
    }
    out = kernel(**inputs)
    print("kernel ran, out shape", out.shape, "std", out.std())


# revision 14
# speedup vs baseline: 1.5213x; 1.1718x over previous
"""Trainium2 Bass kernel for nn_CrossAttention (16x512x64x64, 8 heads x 64).

Math notes (exact algebraic restructuring of the reference):
  The reference tiles ky=[b,1,1,c] to k=[b,c,1,c] before conv1x1(to_k_w), so
  every input channel of that conv carries the same value ky[b,j].  Hence
    conv1x1(k, to_k_w)[b,o,0,j] = rowsum(to_k_w)[o] * ky[b,j]     (rank-1)
  and likewise for v with rowsum(to_v_w) and vy.  Propagating this:
    ksm[b,hd,j] = softmax_j(rs_k[hd] * ky[b,j])
    w[b,hd]     = sum_j ksm[b,hd,j] * vy[b,j]
    s[b,h,n]    = num/den,  num = sum_d w[hd] e^{q[hd,n]}, den = sum_d e^{q[hd,n]}
    mm[b,o,n]   = sum_h W2[o,h] * s[b,h,n] + out_b[o],
      with W2[o,h] = scale * sum_e out_w[o, h*64+e] * rs_v[h*64+e]
  followed by GroupNorm(1) over (C,H,W) per sample:
    out = A[o]*mmW2[o,n] + B[o],  A = gn_g*rstd, B = A*(out_b-mu)+gn_b
  GN stats come from the 9x9 Gram matrix of [s; 1] over n:
    sum mm   = sum_ab Cm[a,b] S2[a,b],   sum mm^2 = sum_ab Gm[a,b] S2[a,b]
  where S2 = [s;1][s;1]^T (accumulated on PE), Cm/Gm folded on host from
  W2 / out_b.

Device layout: q kept transposed [he, n] so the d-softmax reductions are
small PE matmuls (block-diagonal masks carrying w), not DVE reductions.
The only large compute is the q GEMM (to_q_w @ x, bf16, 2.1 GFLOP/sample).

Sharding: data-parallel over batch, 2 samples per core, 8 cores, no
collectives.  Weight folding (transposes, rowsums, W2, Gm/Cm) is done on
host; x is staged to bf16 on host.
"""

import numpy as np
import ml_dtypes

import concourse.bass as bass
import concourse.mybir as mybir
import concourse.tile as tile
from concourse import bacc
from concourse.bass import ts
from concourse.bass_utils import run_bass_kernel_spmd

B, C, N = 16, 512, 4096
DIMY = 768
HEADS, DHEAD = 8, 64
NCORES = 8
BPC = B // NCORES  # samples per core
SCALE = DHEAD ** -0.5
EPS = 1e-5
F32 = mybir.dt.float32
BF16 = mybir.dt.bfloat16
AX = mybir.AxisListType.X
AF = mybir.ActivationFunctionType
OP = mybir.AluOpType
NG = 8          # n-groups of 512 per sample
GSZ = N // NG   # 512
CN = C * N

BF = ml_dtypes.bfloat16


def build_nc():
    nc = bacc.Bacc()
    xd = nc.dram_tensor("x", [BPC, 128, 4, N], BF16, kind="ExternalInput")
    qwTd = nc.dram_tensor("qwT", [128, 4, C], BF16, kind="ExternalInput")
    kvTd = nc.dram_tensor("kvT", [128, 6, 2 * C], BF16, kind="ExternalInput")
    yTd = nc.dram_tensor("yT", [128, 6, BPC], BF16, kind="ExternalInput")
    rskbd = nc.dram_tensor("rskb", [128, C], F32, kind="ExternalInput")
    omaskd = nc.dram_tensor("omask", [128, 4, HEADS], BF16, kind="ExternalInput")
    w2Td = nc.dram_tensor("w2T", [HEADS, 4, 128], BF16, kind="ExternalInput")
    gcmd = nc.dram_tensor("gcm", [9, 2, 9], F32, kind="ExternalInput")
    colsd = nc.dram_tensor("cols", [128, 12], F32, kind="ExternalInput")
    outd = nc.dram_tensor("out", [BPC, 4, 128, N], BF16, kind="ExternalOutput")

    from contextlib import ExitStack

    with tile.TileContext(nc) as tc, ExitStack() as ctx:
        persist = ctx.enter_context(tc.tile_pool(name="persist", bufs=1))
        xp = ctx.enter_context(tc.tile_pool(name="xp", bufs=2))
        ep = ctx.enter_context(tc.tile_pool(name="ep", bufs=3))
        stgp = ctx.enter_context(tc.tile_pool(name="stgp", bufs=4))
        samp = ctx.enter_context(tc.tile_pool(name="samp", bufs=2))
        gp = ctx.enter_context(tc.tile_pool(name="gp", bufs=3))
        tiny = ctx.enter_context(tc.tile_pool(name="tiny", bufs=4))
        psqp = ctx.enter_context(tc.tile_pool(name="psqp", bufs=3, space="PSUM"))
        psndp = ctx.enter_context(tc.tile_pool(name="psndp", bufs=2, space="PSUM"))
        psgp = ctx.enter_context(tc.tile_pool(name="psgp", bufs=2, space="PSUM"))
        pssm = ctx.enter_context(tc.tile_pool(name="pssm", bufs=1, space="PSUM"))

        # ---------------- prep: weights + constants ----------------
        qwT = persist.tile([128, 4, C], BF16, tag="qwT")
        nc.sync.dma_start(out=qwT, in_=qwTd[:, :, :])
        kvT = persist.tile([128, 6, 2 * C], BF16, tag="kvT")
        nc.sync.dma_start(out=kvT, in_=kvTd[:, :, :])
        yT = persist.tile([128, 6, BPC], BF16, tag="yT")
        nc.sync.dma_start(out=yT, in_=yTd[:, :, :])
        rskb = persist.tile([128, C], F32, tag="rskb")
        nc.sync.dma_start(out=rskb, in_=rskbd[:, :])
        omask = persist.tile([128, 4, HEADS], BF16, tag="omask")
        nc.sync.dma_start(out=omask, in_=omaskd[:, :, :])
        w2T = persist.tile([HEADS, 4, 128], BF16, tag="w2T")
        nc.sync.dma_start(out=w2T, in_=w2Td[:, :, :])
        gcm = persist.tile([9, 2, 9], F32, tag="gcm")
        nc.sync.dma_start(out=gcm, in_=gcmd[:, :, :])
        gcols = persist.tile([128, 12], F32, tag="gcols")
        nc.sync.dma_start(out=gcols, in_=colsd[:, :])

        ident = persist.tile([128, 128], F32, tag="ident")
        from concourse.masks import make_identity

        make_identity(nc, ident)
        identb = persist.tile([128, 128], BF16, tag="identb")
        make_identity(nc, identb)
        ones_row = persist.tile([1, 128], F32, tag="onesr")
        nc.vector.memset(ones_row, 1.0)
        ones9 = persist.tile([9, 1], F32, tag="ones9")
        nc.vector.memset(ones9, 1.0)
        zero_col = persist.tile([128, 1], F32, tag="zero")
        nc.vector.memset(zero_col, 0.0)
        nc.const_aps.aps[(F32, 0.0)] = zero_col[:, :]
        eps_col = persist.tile([128, 1], F32, tag="eps")
        nc.vector.memset(eps_col, EPS)
        nc.const_aps.aps[(F32, EPS)] = eps_col[:, :]
        # gram staging: [128 n, 4 j, 9]; col 8 of each j-block stays 1.0
        gstage = persist.tile([128, 4, 9], BF16, tag="gstage")
        nc.vector.memset(gstage[:, :, 8:9], 1.0)
        # s values for both samples: [8 h, s, n]
        s_all = persist.tile([HEADS, BPC, N], BF16, tag="s_all")

        # x for sample 0 (scalar-engine queue), sample 1 (sync queue)
        xts = []
        for s in range(BPC):
            xt = xp.tile([128, 4, N], BF16, tag="xt")
            eng = nc.scalar if s == 0 else nc.sync
            eng.dma_start(out=xt, in_=xd[s])
            xts.append(xt)

        # ---------------- ky/vy for both samples ----------------
        # ky[s, o] = sum_d y[s, d] k_w[o, d]; vy likewise (both via PE)
        krows = tiny.tile([BPC, 2, C], F32, tag="krows")
        for kv in range(2):
            ps_ky = pssm.tile([BPC, C], F32, tag="sm")
            for dt_ in range(6):
                nc.tensor.matmul(
                    ps_ky, lhsT=yT[:, dt_, :], rhs=kvT[:, dt_, kv * C : (kv + 1) * C],
                    start=(dt_ == 0), stop=(dt_ == 5),
                )
            nc.vector.tensor_copy(out=krows[:, kv, :], in_=ps_ky)
        # transpose to columns: kv_cols[p, 4*ot + 2*kv + s]
        ps_kc = pssm.tile([128, 16], F32, tag="sm")
        for ot in range(4):
            for kv in range(2):
                nc.tensor.transpose(
                    ps_kc[:, 4 * ot + 2 * kv : 4 * ot + 2 * kv + 2],
                    krows[:, kv, ts(ot, 128)],
                    ident[0:BPC, 0:BPC],
                )
        kv_cols = persist.tile([128, 16], F32, tag="kvcols")
        nc.vector.tensor_copy(out=kv_cols, in_=ps_kc)

        per_sample = []  # (numq, ab) tiles
        for s in range(BPC):
            # ---- k-softmax -> w ----
            # E_T[j, hd] = exp(ky[j] * rs_k[hd])
            et = ep.tile([128, 4, C], BF16, tag="eq")
            for jt in range(4):
                nc.scalar.activation(
                    out=et[:, jt, :], in_=rskb, func=AF.Exp,
                    scale=kv_cols[:, 4 * jt + s : 4 * jt + s + 1],
                )
            # masks: col 0 = vy (num), col 32 = 1 (den at psum partition 32)
            kvm = tiny.tile([128, 4, 33], BF16, tag="kvm")
            nc.vector.memset(kvm, 0.0)
            nc.vector.tensor_copy(
                out=kvm[:, :, 0:1],
                in_=kv_cols.rearrange("p (a r) -> p a r", r=4)[:, :, 2 + s : 3 + s],
            )
            nc.vector.memset(kvm[:, :, 32:33], 1.0)
            ps_w = pssm.tile([33, C], F32, tag="sm")
            for jt in range(4):
                nc.tensor.matmul(
                    ps_w, lhsT=kvm[:, jt, :], rhs=et[:, jt, :],
                    start=(jt == 0), stop=(jt == 3),
                )
            rdw = tiny.tile([1, C], F32, tag="rdw")
            nc.vector.reciprocal(rdw, ps_w[32:33, :])
            w_row = tiny.tile([1, C], F32, tag="wrow")
            nc.vector.tensor_mul(w_row, ps_w[0:1, :], rdw)
            ps_wc = pssm.tile([128, 4], F32, tag="sm")
            for ht in range(4):
                nc.tensor.transpose(
                    ps_wc[:, ht : ht + 1], w_row[:, ts(ht, 128)], ident[0:1, 0:1]
                )
            w_col = tiny.tile([128, 4], F32, tag="wcol")
            nc.vector.tensor_copy(out=w_col, in_=ps_wc)
            # numq masks: cols 0:8 = omask * w (num), cols 32:40 = omask (den)
            # (den lands at psum partitions 32-39 so the reciprocal's read
            #  stays 32-aligned)
            numq = samp.tile([128, 4, 48], BF16, tag="numq")
            nc.vector.memset(numq, 0.0)
            for ht in range(4):
                nc.vector.tensor_scalar(
                    out=numq[:, ht, 0:HEADS], in0=omask[:, ht, :],
                    scalar1=w_col[:, ht : ht + 1], scalar2=None, op0=OP.mult,
                )
            nc.vector.tensor_copy(out=numq[:, :, 32:40], in_=omask)

            # ---- pass 1: q GEMM + softmax-reduce via PE, Gram deferred ----
            # Groups are paired into one [112, GSZ] psum tile (even group at
            # partition 0, odd at 64) so one reciprocal serves two groups.
            # Transposes/gram for group g are emitted at g+2 so the PE queue
            # never stalls on the DVE reciprocal chain.
            psg = psgp.tile([9, 4, 9], F32, tag="gram")

            def emit_gram(s, g):
                ps_sT = pssm.tile([128, 4 * HEADS], BF16, tag="sm")
                for j in range(4):
                    nc.tensor.transpose(
                        ps_sT[:, 8 * j : 8 * j + 8],
                        s_all[:, s, g * GSZ + 128 * j : g * GSZ + 128 * (j + 1)],
                        identb[0:HEADS, 0:HEADS],
                    )
                nc.vector.tensor_copy(
                    out=gstage[:, :, 0:HEADS],
                    in_=ps_sT.rearrange("p (j h) -> p j h", h=HEADS),
                )
                for j in range(4):
                    nc.tensor.matmul(
                        psg[:, j, :], lhsT=gstage[:, j, :], rhs=gstage[:, j, :],
                        start=(g == 0), stop=(g == NG - 1), skip_group_check=True,
                    )

            psnd = None
            for g in range(NG):
                eq = ep.tile([128, 4, C], BF16, tag="eq")
                for ht in range(4):
                    psq = psqp.tile([128, GSZ], F32, tag="psq")
                    for ct in range(4):
                        nc.tensor.matmul(
                            psq,
                            lhsT=qwT[:, ct, ts(ht, 128)],
                            rhs=xts[s][:, ct, ts(g, GSZ)],
                            start=(ct == 0), stop=(ct == 3),
                        )
                    nc.scalar.activation(out=eq[:, ht, :], in_=psq, func=AF.Exp)
                if g % 2 == 0:
                    psnd = psndp.tile([112, GSZ], F32, tag="nd")
                base = 64 * (g % 2)
                for ht in range(4):
                    nc.tensor.matmul(
                        psnd[base : base + 48, :],
                        lhsT=numq[:, ht, :], rhs=eq[:, ht, :],
                        start=(ht == 0), stop=(ht == 3),
                    )
                if g % 2 == 1:
                    rdf = gp.tile([112, GSZ], F32, tag="rden")
                    nc.vector.reciprocal(rdf, psnd)
                    nc.vector.tensor_mul(
                        s_all[:, s, ts(g - 1, GSZ)], psnd[0:HEADS, :],
                        rdf[32:40, :],
                    )
                    nc.vector.tensor_mul(
                        s_all[:, s, ts(g, GSZ)], psnd[64 : 64 + HEADS, :],
                        rdf[96:104, :],
                    )
                if g >= 3 and g % 2 == 1:
                    emit_gram(s, g - 3)
                    emit_gram(s, g - 2)
            emit_gram(s, NG - 2)
            emit_gram(s, NG - 1)
            per_sample.append((numq, psg))

        # ---- GN stats from Gram (both samples; ACT funcs clustered so the
        # Ln/Exp table sets load at most once each) ----
        stat_tiles = []
        for s in range(BPC):
            psg = per_sample[s][1]
            gsb = tiny.tile([9, 4, 9], F32, tag="gsb")
            nc.vector.tensor_copy(out=gsb, in_=psg)
            s2 = tiny.tile([9, 9], F32, tag="s2")
            nc.vector.reduce_sum(
                out=s2, in_=gsb.rearrange("p j b -> p b j"), axis=AX
            )
            work = tiny.tile([9, 2, 9], F32, tag="work")
            nc.vector.tensor_mul(work[:, 0, :], gcm[:, 0, :], s2)
            nc.vector.tensor_mul(work[:, 1, :], gcm[:, 1, :], s2)
            wred = tiny.tile([9, 2], F32, tag="wred")
            nc.vector.reduce_sum(out=wred, in_=work, axis=AX)
            ps_s = pssm.tile([1, 2], F32, tag="sm")
            nc.tensor.matmul(ps_s, lhsT=ones9, rhs=wred, start=True, stop=True)
            msc = tiny.tile([1, 4], F32, tag="msc")
            nc.scalar.mul(out=msc[:, 0:2], in_=ps_s, mul=1.0 / CN)
            nc.vector.tensor_mul(msc[:, 2:3], msc[:, 0:1], msc[:, 0:1])
            nc.vector.tensor_sub(msc[:, 3:4], msc[:, 1:2], msc[:, 2:3])
            stat_tiles.append(msc)
        lnvs = []
        for s in range(BPC):
            lnv = tiny.tile([1, 2], F32, tag="lnv")
            nc.scalar.activation(
                out=lnv[:, 0:1], in_=stat_tiles[s][:, 3:4], func=AF.Ln, bias=EPS
            )
            lnvs.append(lnv)
        for s in range(BPC):
            nc.scalar.activation(
                out=lnvs[s][:, 1:2], in_=lnvs[s][:, 0:1], func=AF.Exp, scale=-0.5
            )
        abs_ = []
        for s in range(BPC):
            msc, lnv = stat_tiles[s], lnvs[s]
            murow = tiny.tile([1, 2], F32, tag="murow")
            nc.vector.tensor_copy(out=murow[:, 0:1], in_=msc[:, 0:1])
            nc.vector.tensor_copy(out=murow[:, 1:2], in_=lnv[:, 1:2])
            ps_b = pssm.tile([128, 2], F32, tag="sm")
            nc.tensor.matmul(ps_b, lhsT=ones_row, rhs=murow, start=True, stop=True)
            msb = tiny.tile([128, 2], F32, tag="msb")
            nc.vector.tensor_copy(out=msb, in_=ps_b)
            # A = gn_g * rstd ; B = A*(out_b - mu) + gn_b
            ab = samp.tile([128, 2, 4], F32, tag="ab")
            nc.vector.tensor_scalar(
                out=ab[:, 0, :], in0=gcols[:, 0:4],
                scalar1=msb[:, 1:2], scalar2=None, op0=OP.mult,
            )
            t1 = tiny.tile([128, 2, 4], F32, tag="t1")
            nc.vector.tensor_scalar(
                out=t1[:, 0, :], in0=gcols[:, 8:12],
                scalar1=msb[:, 0:1], scalar2=None, op0=OP.subtract,
            )
            nc.vector.tensor_mul(t1[:, 1, :], ab[:, 0, :], t1[:, 0, :])
            nc.vector.tensor_add(ab[:, 1, :], t1[:, 1, :], gcols[:, 4:8])
            abs_.append(ab)

        # ---------------- pass 2: mm = W2 @ s, affine, store ----------------
        # stg copies alternate DVE / ACT so neither engine gates the PE.
        for s in range(BPC):
            ab = abs_[s]
            for g in range(NG):
                for ot in range(4):
                    psf = psqp.tile([128, GSZ], F32, tag="psq")
                    nc.tensor.matmul(
                        psf, lhsT=w2T[:, ot, :], rhs=s_all[:, s, ts(g, GSZ)],
                        start=True, stop=True,
                    )
                    stg = stgp.tile([128, GSZ], BF16, tag="stg")
                    if (g * 4 + ot) % 2 == 0:
                        nc.vector.tensor_scalar(
                            out=stg, in0=psf,
                            scalar1=ab[:, 0, ot : ot + 1],
                            scalar2=ab[:, 1, ot : ot + 1],
                            op0=OP.mult, op1=OP.add,
                        )
                    else:
                        nc.scalar.activation(
                            out=stg, in_=psf, func=AF.Identity,
                            scale=ab[:, 0, ot : ot + 1],
                            bias=ab[:, 1, ot : ot + 1],
                        )
                    nc.sync.dma_start(out=outd[s, ot, :, ts(g, GSZ)], in_=stg)

    nc.finalize()
    return nc


_NC_CACHE = {}


def _get_nc():
    if "nc" not in _NC_CACHE:
        _NC_CACHE["nc"] = build_nc()
    return _NC_CACHE["nc"]


def _fold_host(inputs):
    """Host-side weight folding + staging (shared across cores)."""
    k_w = np.asarray(inputs["k_w"], np.float32)
    v_w = np.asarray(inputs["v_w"], np.float32)
    to_q_w = np.asarray(inputs["to_q_w"], np.float32)
    to_k_w = np.asarray(inputs["to_k_w"], np.float32)
    to_v_w = np.asarray(inputs["to_v_w"], np.float32)
    out_w = np.asarray(inputs["out_w"], np.float32)
    out_b = np.asarray(inputs["out_b"], np.float32)
    gn_g = np.asarray(inputs["gn_g"], np.float32)
    gn_b = np.asarray(inputs["gn_b"], np.float32)

    qwT = np.ascontiguousarray(
        to_q_w.T.reshape(4, 128, C).transpose(1, 0, 2)
    ).astype(BF)  # [128, ct, he]
    kT = k_w.T.reshape(6, 128, C).transpose(1, 0, 2)  # [128, dt, o]
    vT = v_w.T.reshape(6, 128, C).transpose(1, 0, 2)
    kvT = np.ascontiguousarray(np.concatenate([kT, vT], axis=2)).astype(BF)

    rs_k = to_k_w.sum(axis=1)  # [C]
    rs_v = to_v_w.sum(axis=1)
    rskb = np.ascontiguousarray(np.broadcast_to(rs_k[None, :], (128, C))).astype(
        np.float32
    )

    # W2[o, h] = scale * sum_e out_w[o, h*64+e] * rs_v[h*64+e]
    W2 = SCALE * np.einsum(
        "ohe,he->oh", out_w.reshape(C, HEADS, DHEAD), rs_v.reshape(HEADS, DHEAD)
    )  # [C, HEADS]
    w2T = np.ascontiguousarray(
        W2.reshape(4, 128, HEADS).transpose(2, 0, 1)
    ).astype(BF)  # [h, ot, p]

    # Gm/Cm: sum mm^q = sum_ab M[a,b] S2[a,b], S2 = [s;1][s;1]^T over n
    G = W2.T @ W2  # [8, 8]
    colsumW2 = W2.sum(axis=0)  # [8]
    bW2 = out_b @ W2  # [8]
    Gm = np.zeros((9, 9), np.float32)
    Gm[:8, :8] = G
    Gm[8, :8] = bW2
    Gm[:8, 8] = bW2
    Gm[8, 8] = float((out_b ** 2).sum())
    Cm = np.zeros((9, 9), np.float32)
    Cm[8, :8] = colsumW2 / 2.0
    Cm[:8, 8] = colsumW2 / 2.0
    Cm[8, 8] = float(out_b.sum())
    gcm = np.ascontiguousarray(
        np.stack([Cm, Gm], axis=1)
    ).astype(np.float32)  # [9, 2, 9]

    omask = np.zeros((128, 4, HEADS), np.float32)
    for ht in range(4):
        for p in range(128):
            omask[p, ht, 2 * ht + p // 64] = 1.0
    omask = omask.astype(BF)

    cols = np.ascontiguousarray(
        np.stack(
            [*gn_g.reshape(4, 128), *gn_b.reshape(4, 128), *out_b.reshape(4, 128)],
            axis=1,
        )
    ).astype(np.float32)  # [128, 12]
    return dict(qwT=qwT, kvT=kvT, rskb=rskb, omask=omask, w2T=w2T, gcm=gcm, cols=cols)


def make_in_maps(inputs):
    x = np.asarray(inputs["x"], np.float32).reshape(B, 4, 128, N)
    x = np.ascontiguousarray(x).astype(BF)
    y = np.asarray(inputs["y"], np.float32).reshape(B, DIMY)
    shared = _fold_host(inputs)
    in_maps = []
    for core in range(NCORES):
        s0 = core * BPC
        yc = y[s0 : s0 + BPC]  # [BPC, DIMY]
        yT = np.ascontiguousarray(
            yc.T.reshape(6, 128, BPC).transpose(1, 0, 2)
        ).astype(BF)
        m = {"x": x[s0 : s0 + BPC].transpose(0, 2, 1, 3).copy(), "yT": yT}
        m.update(shared)
        in_maps.append(m)
    return in_maps


def kernel(**inputs):
    nc = _get_nc()
    res = run_bass_kernel_spmd(nc, make_in_maps(inputs), list(range(NCORES)))
    out = np.concatenate([r["out"] for r in res.results], axis=0)  # [B, 4, 128, N] bf16
    return out.astype(np.float32).reshape(B, C, 64, 64)


if __name__ == "__main__":
    rng = np.random.default_rng(0)
    inputs = {
        "x": rng.standard_normal((B, C, 64, 64), dtype=np.float32),
        "y": rng.standard_normal((B, 1, 1, DIMY), dtype=np.float32),
        "k_w": rng.standard_normal((C, DIMY), dtype=np.float32) * 0.02,
        "v_w": rng.standard_normal((C, DIMY), dtype=np.float32) * 0.02,
        "to_q_w": rng.standard_normal((C, C), dtype=np.float32) * 0.02,
        "to_k_w": rng.standard_normal((C, C), dtype=np.float32) * 0.02,
        "to_v_w": rng.standard_normal((C, C), dtype=np.float32) * 0.02,
        "out_w": rng.standard_normal((C, C), dtype=np.float32) * 0.02,
        "out_b": np.zeros(C, np.float32),
        "gn_g": np.ones(C, np.float32),
        "gn_b": np.zeros(C, np.float32),
    }
    out = kernel(**inputs)
    print("kernel ran, out shape", out.shape, "std", out.std())


# revision 18
# speedup vs baseline: 1.7593x; 1.1564x over previous
"""Trainium2 Bass kernel for nn_CrossAttention (16x512x64x64, 8 heads x 64).

Math notes (exact algebraic restructuring of the reference):
  The reference tiles ky=[b,1,1,c] to k=[b,c,1,c] before conv1x1(to_k_w), so
  every input channel of that conv carries the same value ky[b,j].  Hence
    conv1x1(k, to_k_w)[b,o,0,j] = rowsum(to_k_w)[o] * ky[b,j]     (rank-1)
  and likewise for v with rowsum(to_v_w) and vy.  Propagating this:
    ksm[b,hd,j] = softmax_j(rs_k[hd] * ky[b,j])
    w[b,hd]     = sum_j ksm[b,hd,j] * vy[b,j]
    s[b,h,n]    = num/den,  num = sum_d w[hd] e^{q[hd,n]}, den = sum_d e^{q[hd,n]}
    mm[b,o,n]   = sum_h W2[o,h] * s[b,h,n] + out_b[o],
      with W2[o,h] = scale * sum_e out_w[o, h*64+e] * rs_v[h*64+e]
  followed by GroupNorm(1) over (C,H,W) per sample:
    out = A[o]*mmW2[o,n] + B[o],  A = gn_g*rstd, B = A*(out_b-mu)+gn_b
  GN stats come from the 9x9 Gram matrix of [s; 1] over n:
    sum mm   = sum_ab Cm[a,b] S2[a,b],   sum mm^2 = sum_ab Gm[a,b] S2[a,b]
  where S2 = [s;1][s;1]^T (accumulated on PE), Cm/Gm folded on host from
  W2 / out_b.

Device layout: q kept transposed [he, n] so the d-softmax reductions are
small PE matmuls (block-diagonal masks carrying w), not DVE reductions.
The only large compute is the q GEMM (to_q_w @ x, bf16, 2.1 GFLOP/sample).

Sharding: data-parallel over batch, 2 samples per core, 8 cores, no
collectives.  Weight folding (transposes, rowsums, W2, Gm/Cm) is done on
host; x is staged to bf16 on host.
"""

import numpy as np
import ml_dtypes

import concourse.bass as bass
import concourse.mybir as mybir
import concourse.tile as tile
from concourse import bacc
from concourse.bass import ts
from concourse.bass_utils import run_bass_kernel_spmd

B, C, N = 16, 512, 4096
DIMY = 768
HEADS, DHEAD = 8, 64
NCORES = 8
BPC = B // NCORES  # samples per core
SCALE = DHEAD ** -0.5
EPS = 1e-5
F32 = mybir.dt.float32
BF16 = mybir.dt.bfloat16
AX = mybir.AxisListType.X
AF = mybir.ActivationFunctionType
OP = mybir.AluOpType
NG = 8          # n-groups of 512 per sample
GSZ = N // NG   # 512
CN = C * N

BF = ml_dtypes.bfloat16


def build_nc():
    nc = bacc.Bacc()
    xd = nc.dram_tensor("x", [BPC, 128, 4, N], BF16, kind="ExternalInput")
    qwTd = nc.dram_tensor("qwT", [128, 4, C], BF16, kind="ExternalInput")
    kvTd = nc.dram_tensor("kvT", [128, 6, 2 * C], BF16, kind="ExternalInput")
    yTd = nc.dram_tensor("yT", [128, 6, BPC], BF16, kind="ExternalInput")
    rskbd = nc.dram_tensor("rskb", [128, C], F32, kind="ExternalInput")
    omaskd = nc.dram_tensor("omask", [128, 4, HEADS], BF16, kind="ExternalInput")
    w2Td = nc.dram_tensor("w2T", [HEADS, 4, 128], BF16, kind="ExternalInput")
    gcmd = nc.dram_tensor("gcm", [9, 2, 9], F32, kind="ExternalInput")
    colsd = nc.dram_tensor("cols", [128, 12], F32, kind="ExternalInput")
    outd = nc.dram_tensor("out", [BPC, 4, 128, N], BF16, kind="ExternalOutput")

    from contextlib import ExitStack

    with tile.TileContext(nc) as tc, ExitStack() as ctx:
        persist = ctx.enter_context(tc.tile_pool(name="persist", bufs=1))
        xp = ctx.enter_context(tc.tile_pool(name="xp", bufs=2))
        ep = ctx.enter_context(tc.tile_pool(name="ep", bufs=3))
        stgp = ctx.enter_context(tc.tile_pool(name="stgp", bufs=4))
        samp = ctx.enter_context(tc.tile_pool(name="samp", bufs=2))
        gp = ctx.enter_context(tc.tile_pool(name="gp", bufs=3))
        tiny = ctx.enter_context(tc.tile_pool(name="tiny", bufs=4))
        psqp = ctx.enter_context(tc.tile_pool(name="psqp", bufs=2, space="PSUM"))
        psndp = ctx.enter_context(tc.tile_pool(name="psndp", bufs=2, space="PSUM"))
        psf2p = ctx.enter_context(tc.tile_pool(name="psf2p", bufs=2, space="PSUM"))
        psgp = ctx.enter_context(tc.tile_pool(name="psgp", bufs=1, space="PSUM"))
        pssm = ctx.enter_context(tc.tile_pool(name="pssm", bufs=1, space="PSUM"))

        # ---------------- prep: weights + constants ----------------
        qwT = persist.tile([128, 4, C], BF16, tag="qwT")
        nc.sync.dma_start(out=qwT, in_=qwTd[:, :, :])
        kvT = persist.tile([128, 6, 2 * C], BF16, tag="kvT")
        nc.sync.dma_start(out=kvT, in_=kvTd[:, :, :])
        yT = persist.tile([128, 6, BPC], BF16, tag="yT")
        nc.sync.dma_start(out=yT, in_=yTd[:, :, :])
        rskb = persist.tile([128, C], F32, tag="rskb")
        nc.sync.dma_start(out=rskb, in_=rskbd[:, :])
        omask = persist.tile([128, 4, HEADS], BF16, tag="omask")
        nc.sync.dma_start(out=omask, in_=omaskd[:, :, :])
        w2T = persist.tile([HEADS, 4, 128], BF16, tag="w2T")
        nc.sync.dma_start(out=w2T, in_=w2Td[:, :, :])
        gcm = persist.tile([9, 2, 9], F32, tag="gcm")
        nc.sync.dma_start(out=gcm, in_=gcmd[:, :, :])
        gcols = persist.tile([128, 12], F32, tag="gcols")
        nc.sync.dma_start(out=gcols, in_=colsd[:, :])

        ident = persist.tile([128, 128], F32, tag="ident")
        from concourse.masks import make_identity

        make_identity(nc, ident)
        identb = persist.tile([128, 128], BF16, tag="identb")
        make_identity(nc, identb)
        ones_row = persist.tile([1, 128], F32, tag="onesr")
        nc.vector.memset(ones_row, 1.0)
        ones9 = persist.tile([9, 1], F32, tag="ones9")
        nc.vector.memset(ones9, 1.0)
        zero_col = persist.tile([128, 1], F32, tag="zero")
        nc.vector.memset(zero_col, 0.0)
        nc.const_aps.aps[(F32, 0.0)] = zero_col[:, :]
        eps_col = persist.tile([128, 1], F32, tag="eps")
        nc.vector.memset(eps_col, EPS)
        nc.const_aps.aps[(F32, EPS)] = eps_col[:, :]
        # gram staging: [128 n, 4 j, 9]; col 8 of each j-block stays 1.0
        gstage = persist.tile([128, 4, 9], BF16, tag="gstage")
        nc.vector.memset(gstage[:, :, 8:9], 1.0)
        # s values for both samples: [8 h, s, n]
        s_all = persist.tile([HEADS, BPC, N], BF16, tag="s_all")

        # x for sample 0 (scalar-engine queue), sample 1 (sync queue)
        xts = []
        for s in range(BPC):
            xt = xp.tile([128, 4, N], BF16, tag="xt")
            eng = nc.scalar if s == 0 else nc.sync
            eng.dma_start(out=xt, in_=xd[s])
            xts.append(xt)

        # ---------------- ky/vy for both samples ----------------
        # ky[s, o] = sum_d y[s, d] k_w[o, d]; vy likewise (both via PE)
        krows = tiny.tile([BPC, 2, C], F32, tag="krows")
        for kv in range(2):
            ps_ky = pssm.tile([BPC, C], F32, tag="sm")
            for dt_ in range(6):
                nc.tensor.matmul(
                    ps_ky, lhsT=yT[:, dt_, :], rhs=kvT[:, dt_, kv * C : (kv + 1) * C],
                    start=(dt_ == 0), stop=(dt_ == 5),
                )
            nc.vector.tensor_copy(out=krows[:, kv, :], in_=ps_ky)
        # transpose to columns: kv_cols[p, 4*ot + 2*kv + s]
        ps_kc = pssm.tile([128, 16], F32, tag="sm")
        for ot in range(4):
            for kv in range(2):
                nc.tensor.transpose(
                    ps_kc[:, 4 * ot + 2 * kv : 4 * ot + 2 * kv + 2],
                    krows[:, kv, ts(ot, 128)],
                    ident[0:BPC, 0:BPC],
                )
        kv_cols = persist.tile([128, 16], F32, tag="kvcols")
        nc.vector.tensor_copy(out=kv_cols, in_=ps_kc)

        # ---------------- k-softmax -> w, masks (both samples upfront) ----
        numqs = []
        for s in range(BPC):
            # E_T[j, hd] = exp(ky[j] * rs_k[hd])
            et = ep.tile([128, 4, C], BF16, tag="eq")
            for jt in range(4):
                nc.scalar.activation(
                    out=et[:, jt, :], in_=rskb, func=AF.Exp,
                    scale=kv_cols[:, 4 * jt + s : 4 * jt + s + 1],
                )
            # masks: col 0 = vy (num), col 32 = 1 (den at psum partition 32)
            kvm = tiny.tile([128, 4, 33], BF16, tag="kvm")
            nc.vector.memset(kvm, 0.0)
            nc.vector.tensor_copy(
                out=kvm[:, :, 0:1],
                in_=kv_cols.rearrange("p (a r) -> p a r", r=4)[:, :, 2 + s : 3 + s],
            )
            nc.vector.memset(kvm[:, :, 32:33], 1.0)
            ps_w = pssm.tile([33, C], F32, tag="sm")
            for jt in range(4):
                nc.tensor.matmul(
                    ps_w, lhsT=kvm[:, jt, :], rhs=et[:, jt, :],
                    start=(jt == 0), stop=(jt == 3),
                )
            dwsb = tiny.tile([1, 2, C], F32, tag="dwsb")
            nc.vector.tensor_copy(out=dwsb[:, 0, :], in_=ps_w[32:33, :])
            nc.vector.reciprocal_approx_fast(out=dwsb[:, 1, :], in_=dwsb[:, 0, :])
            w_row = tiny.tile([1, C], F32, tag="wrow")
            nc.vector.tensor_mul(w_row, ps_w[0:1, :], dwsb[:, 1, :])
            ps_wc = pssm.tile([128, 4], F32, tag="sm")
            for ht in range(4):
                nc.tensor.transpose(
                    ps_wc[:, ht : ht + 1], w_row[:, ts(ht, 128)], ident[0:1, 0:1]
                )
            w_col = tiny.tile([128, 4], F32, tag="wcol")
            nc.vector.tensor_copy(out=w_col, in_=ps_wc)
            # numq masks: cols 0:8 = omask * w (num), cols 32:40 = omask (den)
            numq = samp.tile([128, 4, 48], BF16, tag="numq")
            nc.vector.memset(numq, 0.0)
            for ht in range(4):
                nc.vector.tensor_scalar(
                    out=numq[:, ht, 0:HEADS], in0=omask[:, ht, :],
                    scalar1=w_col[:, ht : ht + 1], scalar2=None, op0=OP.mult,
                )
            nc.vector.tensor_copy(out=numq[:, :, 32:40], in_=omask)
            numqs.append(numq)

        # ---------------- per-sample pass 1 ----------------
        psgs = {}

        def emit_gram(s, g):
            ps_sT = pssm.tile([128, 4 * HEADS], BF16, tag="sm")
            for j in range(4):
                nc.tensor.transpose(
                    ps_sT[:, 8 * j : 8 * j + 8],
                    s_all[:, s, g * GSZ + 128 * j : g * GSZ + 128 * (j + 1)],
                    identb[0:HEADS, 0:HEADS],
                )
            nc.vector.tensor_copy(
                out=gstage[:, :, 0:HEADS],
                in_=ps_sT.rearrange("p (j h) -> p j h", h=HEADS),
            )
            for j in range(4):
                nc.tensor.matmul(
                    psgs[s][:, j, :], lhsT=gstage[:, j, :], rhs=gstage[:, j, :],
                    start=(g == 0), stop=(g == NG - 1), skip_group_check=True,
                )

        def pass1_group(s, g, psnd_box, extra=None):
            """One n-group of 512: q GEMM, exp, nd matmuls; every odd group
            finishes the pair (reciprocal + muls).  Gram work for groups g-3,
            g-2 is emitted first so the PE never waits on the DVE chain.
            `extra` emits interleaved work (pass2 units of the other sample)."""
            if g >= 3 and g % 2 == 1:
                emit_gram(s, g - 3)
                emit_gram(s, g - 2)
            eq = ep.tile([128, 4, C], BF16, tag="eq")
            for ht in range(4):
                psq = psqp.tile([128, GSZ], F32, tag="psq")
                for ct in range(4):
                    nc.tensor.matmul(
                        psq,
                        lhsT=qwT[:, ct, ts(ht, 128)],
                        rhs=xts[s][:, ct, ts(g, GSZ)],
                        start=(ct == 0), stop=(ct == 3),
                    )
                nc.scalar.activation(out=eq[:, ht, :], in_=psq, func=AF.Exp)
            if g % 2 == 0:
                psnd_box[0] = psndp.tile([112, GSZ], F32, tag="nd", name="psnd")
            psnd = psnd_box[0]
            base = 64 * (g % 2)
            for ht in range(4):
                nc.tensor.matmul(
                    psnd[base : base + 48, :],
                    lhsT=numqs[s][:, ht, :], rhs=eq[:, ht, :],
                    start=(ht == 0), stop=(ht == 3),
                )
            if extra is not None:
                extra()
            if g % 2 == 1:
                dsb = gp.tile([112, GSZ], F32, tag="dsb")
                nc.vector.tensor_copy(out=dsb, in_=psnd)
                rdf = gp.tile([112, GSZ], F32, tag="rden")
                nc.vector.reciprocal_approx_fast(out=rdf, in_=dsb)
                nc.vector.tensor_mul(
                    s_all[:, s, ts(g - 1, GSZ)], psnd[0:HEADS, :], rdf[32:40, :]
                )
                nc.vector.tensor_mul(
                    s_all[:, s, ts(g, GSZ)], psnd[64 : 64 + HEADS, :],
                    rdf[96:104, :],
                )

        def stats(s):
            """GN stats from the Gram psum; DVE-only (bit-trick rsqrt) so the
            ACT exp table never swaps out."""
            psg = psgs[s]
            gsb = tiny.tile([9, 4, 9], F32, tag="gsb")
            nc.vector.tensor_copy(out=gsb, in_=psg)
            s2 = tiny.tile([9, 9], F32, tag="s2")
            nc.vector.reduce_sum(
                out=s2, in_=gsb.rearrange("p j b -> p b j"), axis=AX
            )
            work = tiny.tile([9, 2, 9], F32, tag="work")
            nc.vector.tensor_mul(work[:, 0, :], gcm[:, 0, :], s2)
            nc.vector.tensor_mul(work[:, 1, :], gcm[:, 1, :], s2)
            wred = tiny.tile([9, 2], F32, tag="wred")
            nc.vector.reduce_sum(out=wred, in_=work, axis=AX)
            ps_s = pssm.tile([1, 2], F32, tag="sm")
            nc.tensor.matmul(ps_s, lhsT=ones9, rhs=wred, start=True, stop=True)
            msc = tiny.tile([1, 4], F32, tag="msc")
            nc.vector.tensor_scalar(
                out=msc[:, 0:2], in0=ps_s, scalar1=1.0 / CN, scalar2=None,
                op0=OP.mult,
            )
            nc.vector.tensor_mul(msc[:, 2:3], msc[:, 0:1], msc[:, 0:1])
            nc.vector.tensor_sub(msc[:, 3:4], msc[:, 1:2], msc[:, 2:3])
            # rstd = rsqrt(var + eps): quake seed + 3 Newton steps, all DVE
            nt = tiny.tile([1, 12], F32, tag="nt")
            nc.vector.tensor_scalar(
                out=nt[:, 0:1], in0=msc[:, 3:4], scalar1=EPS, scalar2=None,
                op0=OP.add,
            )
            v = nt[:, 0:1]
            nt_i = nt.bitcast(mybir.dt.int32)
            nc.vector.tensor_scalar(
                out=nt_i[:, 1:2], in0=nt_i[:, 0:1], scalar1=1, scalar2=None,
                op0=OP.arith_shift_right,
            )
            nc.vector.tensor_scalar(
                out=nt_i[:, 2:3], in0=nt_i[:, 1:2], scalar1=-1,
                scalar2=0x5F3759DF, op0=OP.mult, op1=OP.add,
            )
            y = nt[:, 2:3]
            for it in range(3):
                b0 = 3 + 3 * it
                nc.vector.tensor_mul(nt[:, b0 : b0 + 1], y, y)
                nc.vector.tensor_mul(nt[:, b0 + 1 : b0 + 2], nt[:, b0 : b0 + 1], v)
                nc.vector.tensor_scalar(
                    out=nt[:, b0 + 2 : b0 + 3], in0=nt[:, b0 + 1 : b0 + 2],
                    scalar1=-0.5, scalar2=1.5, op0=OP.mult, op1=OP.add,
                )
                ynew = tiny.tile([1, 1], F32, tag=f"yn{it}")
                nc.vector.tensor_mul(ynew, y, nt[:, b0 + 2 : b0 + 3])
                y = ynew
            murow = tiny.tile([1, 2], F32, tag="murow")
            nc.vector.tensor_copy(out=murow[:, 0:1], in_=msc[:, 0:1])
            nc.vector.tensor_copy(out=murow[:, 1:2], in_=y)
            ps_b = pssm.tile([128, 2], F32, tag="sm")
            nc.tensor.matmul(ps_b, lhsT=ones_row, rhs=murow, start=True, stop=True)
            msb = tiny.tile([128, 2], F32, tag="msb")
            nc.vector.tensor_copy(out=msb, in_=ps_b)
            # A = gn_g * rstd ; B = A*(out_b - mu) + gn_b
            ab = samp.tile([128, 2, 4], F32, tag="ab")
            nc.vector.tensor_scalar(
                out=ab[:, 0, :], in0=gcols[:, 0:4],
                scalar1=msb[:, 1:2], scalar2=None, op0=OP.mult,
            )
            t1 = tiny.tile([128, 2, 4], F32, tag="t1")
            nc.vector.tensor_scalar(
                out=t1[:, 0, :], in0=gcols[:, 8:12],
                scalar1=msb[:, 0:1], scalar2=None, op0=OP.subtract,
            )
            nc.vector.tensor_mul(t1[:, 1, :], ab[:, 0, :], t1[:, 0, :])
            nc.vector.tensor_add(ab[:, 1, :], t1[:, 1, :], gcols[:, 4:8])
            return ab

        def pass2_unit(s, g, ot, idx, ab):
            psf = psf2p.tile([128, GSZ], F32, tag="psf")
            nc.tensor.matmul(
                psf, lhsT=w2T[:, ot, :], rhs=s_all[:, s, ts(g, GSZ)],
                start=True, stop=True,
            )
            stg = stgp.tile([128, GSZ], BF16, tag="stg")
            if idx % 2 == 0:
                nc.vector.tensor_scalar(
                    out=stg, in0=psf,
                    scalar1=ab[:, 0, ot : ot + 1], scalar2=ab[:, 1, ot : ot + 1],
                    op0=OP.mult, op1=OP.add,
                )
            else:
                nc.scalar.activation(
                    out=stg, in_=psf, func=AF.Identity,
                    scale=ab[:, 0, ot : ot + 1], bias=ab[:, 1, ot : ot + 1],
                )
            nc.sync.dma_start(out=outd[s, ot, :, ts(g, GSZ)], in_=stg)

        # pass 1 of sample 0
        psgs[0] = psgp.tile([9, 4, 9], F32, tag="gram", name="psg0")
        box = [None]
        for g in range(NG):
            pass1_group(0, g, box)
        emit_gram(0, NG - 2)
        emit_gram(0, NG - 1)
        ab0 = stats(0)

        # pass 1 of sample 1, with sample 0's pass 2 interleaved
        psgs[1] = psgp.tile([9, 4, 9], F32, tag="gram", name="psg1")
        box = [None]
        for g in range(NG):
            def extra(g=g):
                for ot in range(4):
                    pass2_unit(0, g, ot, g * 4 + ot, ab0)
            pass1_group(1, g, box, extra=extra)
        emit_gram(1, NG - 2)
        emit_gram(1, NG - 1)
        ab1 = stats(1)

        # pass 2 of sample 1
        for g in range(NG):
            for ot in range(4):
                pass2_unit(1, g, ot, g * 4 + ot, ab1)

    nc.finalize()
    return nc


_NC_CACHE = {}


def _get_nc():
    if "nc" not in _NC_CACHE:
        _NC_CACHE["nc"] = build_nc()
    return _NC_CACHE["nc"]


def _fold_host(inputs):
    """Host-side weight folding + staging (shared across cores)."""
    k_w = np.asarray(inputs["k_w"], np.float32)
    v_w = np.asarray(inputs["v_w"], np.float32)
    to_q_w = np.asarray(inputs["to_q_w"], np.float32)
    to_k_w = np.asarray(inputs["to_k_w"], np.float32)
    to_v_w = np.asarray(inputs["to_v_w"], np.float32)
    out_w = np.asarray(inputs["out_w"], np.float32)
    out_b = np.asarray(inputs["out_b"], np.float32)
    gn_g = np.asarray(inputs["gn_g"], np.float32)
    gn_b = np.asarray(inputs["gn_b"], np.float32)

    qwT = np.ascontiguousarray(
        to_q_w.T.reshape(4, 128, C).transpose(1, 0, 2)
    ).astype(BF)  # [128, ct, he]
    kT = k_w.T.reshape(6, 128, C).transpose(1, 0, 2)  # [128, dt, o]
    vT = v_w.T.reshape(6, 128, C).transpose(1, 0, 2)
    kvT = np.ascontiguousarray(np.concatenate([kT, vT], axis=2)).astype(BF)

    rs_k = to_k_w.sum(axis=1)  # [C]
    rs_v = to_v_w.sum(axis=1)
    rskb = np.ascontiguousarray(np.broadcast_to(rs_k[None, :], (128, C))).astype(
        np.float32
    )

    # W2[o, h] = scale * sum_e out_w[o, h*64+e] * rs_v[h*64+e]
    W2 = SCALE * np.einsum(
        "ohe,he->oh", out_w.reshape(C, HEADS, DHEAD), rs_v.reshape(HEADS, DHEAD)
    )  # [C, HEADS]
    w2T = np.ascontiguousarray(
        W2.reshape(4, 128, HEADS).transpose(2, 0, 1)
    ).astype(BF)  # [h, ot, p]

    # Gm/Cm: sum mm^q = sum_ab M[a,b] S2[a,b], S2 = [s;1][s;1]^T over n
    G = W2.T @ W2  # [8, 8]
    colsumW2 = W2.sum(axis=0)  # [8]
    bW2 = out_b @ W2  # [8]
    Gm = np.zeros((9, 9), np.float32)
    Gm[:8, :8] = G
    Gm[8, :8] = bW2
    Gm[:8, 8] = bW2
    Gm[8, 8] = float((out_b ** 2).sum())
    Cm = np.zeros((9, 9), np.float32)
    Cm[8, :8] = colsumW2 / 2.0
    Cm[:8, 8] = colsumW2 / 2.0
    Cm[8, 8] = float(out_b.sum())
    gcm = np.ascontiguousarray(
        np.stack([Cm, Gm], axis=1)
    ).astype(np.float32)  # [9, 2, 9]

    omask = np.zeros((128, 4, HEADS), np.float32)
    for ht in range(4):
        for p in range(128):
            omask[p, ht, 2 * ht + p // 64] = 1.0
    omask = omask.astype(BF)

    cols = np.ascontiguousarray(
        np.stack(
            [*gn_g.reshape(4, 128), *gn_b.reshape(4, 128), *out_b.reshape(4, 128)],
            axis=1,
        )
    ).astype(np.float32)  # [128, 12]
    return dict(qwT=qwT, kvT=kvT, rskb=rskb, omask=omask, w2T=w2T, gcm=gcm, cols=cols)


def make_in_maps(inputs):
    x = np.asarray(inputs["x"], np.float32).reshape(B, 4, 128, N)
    x = np.ascontiguousarray(x).astype(BF)
    y = np.asarray(inputs["y"], np.float32).reshape(B, DIMY)
    shared = _fold_host(inputs)
    in_maps = []
    for core in range(NCORES):
        s0 = core * BPC
        yc = y[s0 : s0 + BPC]  # [BPC, DIMY]
        yT = np.ascontiguousarray(
            yc.T.reshape(6, 128, BPC).transpose(1, 0, 2)
        ).astype(BF)
        m = {"x": x[s0 : s0 + BPC].transpose(0, 2, 1, 3).copy(), "yT": yT}
        m.update(shared)
        in_maps.append(m)
    return in_maps


def kernel(**inputs):
    nc = _get_nc()
    res = run_bass_kernel_spmd(nc, make_in_maps(inputs), list(range(NCORES)))
    out = np.concatenate([r["out"] for r in res.results], axis=0)  # [B, 4, 128, N] bf16
    return out.astype(np.float32).reshape(B, C, 64, 64)


if __name__ == "__main__":
    rng = np.random.default_rng(0)
    inputs = {
        "x": rng.standard_normal((B, C, 64, 64), dtype=np.float32),
        "y": rng.standard_normal((B, 1, 1, DIMY), dtype=np.float32),
        "k_w": rng.standard_normal((C, DIMY), dtype=np.float32) * 0.02,
        "v_w": rng.standard_normal((C, DIMY), dtype=np.float32) * 0.02,
        "to_q_w": rng.standard_normal((C, C), dtype=np.float32) * 0.02,
        "to_k_w": rng.standard_normal((C, C), dtype=np.float32) * 0.02,
        "to_v_w": rng.standard_normal((C, C), dtype=np.float32) * 0.02,
        "out_w": rng.standard_normal((C, C), dtype=np.float32) * 0.02,
        "out_b": np.zeros(C, np.float32),
        "gn_g": np.ones(C, np.float32),
        "gn_b": np.zeros(C, np.float32),
    }
    out = kernel(**inputs)
    print("kernel ran, out shape", out.shape, "std", out.std())


# revision 19
# speedup vs baseline: 1.8519x; 1.0526x over previous
"""Trainium2 Bass kernel for nn_CrossAttention (16x512x64x64, 8 heads x 64).

Math notes (exact algebraic restructuring of the reference):
  The reference tiles ky=[b,1,1,c] to k=[b,c,1,c] before conv1x1(to_k_w), so
  every input channel of that conv carries the same value ky[b,j].  Hence
    conv1x1(k, to_k_w)[b,o,0,j] = rowsum(to_k_w)[o] * ky[b,j]     (rank-1)
  and likewise for v with rowsum(to_v_w) and vy.  Propagating this:
    ksm[b,hd,j] = softmax_j(rs_k[hd] * ky[b,j])
    w[b,hd]     = sum_j ksm[b,hd,j] * vy[b,j]
    s[b,h,n]    = num/den,  num = sum_d w[hd] e^{q[hd,n]}, den = sum_d e^{q[hd,n]}
    mm[b,o,n]   = sum_h W2[o,h] * s[b,h,n] + out_b[o],
      with W2[o,h] = scale * sum_e out_w[o, h*64+e] * rs_v[h*64+e]
  followed by GroupNorm(1) over (C,H,W) per sample:
    out = A[o]*mmW2[o,n] + B[o],  A = gn_g*rstd, B = A*(out_b-mu)+gn_b
  GN stats come from the 9x9 Gram matrix of [s; 1] over n:
    sum mm   = sum_ab Cm[a,b] S2[a,b],   sum mm^2 = sum_ab Gm[a,b] S2[a,b]
  where S2 = [s;1][s;1]^T (accumulated on PE), Cm/Gm folded on host from
  W2 / out_b.

Device layout: q kept transposed [he, n] so the d-softmax reductions are
small PE matmuls (block-diagonal masks carrying w), not DVE reductions.
The only large compute is the q GEMM (to_q_w @ x, bf16, 2.1 GFLOP/sample).

Sharding: data-parallel over batch, 2 samples per core, 8 cores, no
collectives.  Weight folding (transposes, rowsums, W2, Gm/Cm) is done on
host; x is staged to bf16 on host.
"""

import numpy as np
import ml_dtypes

import concourse.bass as bass
import concourse.mybir as mybir
import concourse.tile as tile
from concourse import bacc
from concourse.bass import ts
from concourse.bass_utils import run_bass_kernel_spmd

B, C, N = 16, 512, 4096
DIMY = 768
HEADS, DHEAD = 8, 64
NCORES = 8
BPC = B // NCORES  # samples per core
SCALE = DHEAD ** -0.5
EPS = 1e-5
F32 = mybir.dt.float32
BF16 = mybir.dt.bfloat16
AX = mybir.AxisListType.X
AF = mybir.ActivationFunctionType
OP = mybir.AluOpType
NG = 8          # n-groups of 512 per sample
GSZ = N // NG   # 512
CN = C * N

BF = ml_dtypes.bfloat16


def build_nc():
    nc = bacc.Bacc()
    xd = nc.dram_tensor("x", [BPC, 128, 4, N], BF16, kind="ExternalInput")
    qwTd = nc.dram_tensor("qwT", [128, 4, C], BF16, kind="ExternalInput")
    kvTd = nc.dram_tensor("kvT", [128, 6, 2 * C], BF16, kind="ExternalInput")
    yTd = nc.dram_tensor("yT", [128, 6, BPC], BF16, kind="ExternalInput")
    rskbd = nc.dram_tensor("rskb", [128, C], F32, kind="ExternalInput")
    omaskd = nc.dram_tensor("omask", [128, 4, HEADS], BF16, kind="ExternalInput")
    w2Td = nc.dram_tensor("w2T", [HEADS, 4, 128], BF16, kind="ExternalInput")
    gcmd = nc.dram_tensor("gcm", [9, 2, 9], F32, kind="ExternalInput")
    colsd = nc.dram_tensor("cols", [128, 12], F32, kind="ExternalInput")
    outd = nc.dram_tensor("out", [BPC, 4, 128, N], BF16, kind="ExternalOutput")

    from contextlib import ExitStack

    with tile.TileContext(nc) as tc, ExitStack() as ctx:
        persist = ctx.enter_context(tc.tile_pool(name="persist", bufs=1))
        xp = ctx.enter_context(tc.tile_pool(name="xp", bufs=2))
        ep = ctx.enter_context(tc.tile_pool(name="ep", bufs=3))
        stgp = ctx.enter_context(tc.tile_pool(name="stgp", bufs=4))
        samp = ctx.enter_context(tc.tile_pool(name="samp", bufs=2))
        gp = ctx.enter_context(tc.tile_pool(name="gp", bufs=3))
        tiny = ctx.enter_context(tc.tile_pool(name="tiny", bufs=4))
        psqp = ctx.enter_context(tc.tile_pool(name="psqp", bufs=2, space="PSUM"))
        psndp = ctx.enter_context(tc.tile_pool(name="psndp", bufs=2, space="PSUM"))
        psf2p = ctx.enter_context(tc.tile_pool(name="psf2p", bufs=2, space="PSUM"))
        psgp = ctx.enter_context(tc.tile_pool(name="psgp", bufs=1, space="PSUM"))
        pssm = ctx.enter_context(tc.tile_pool(name="pssm", bufs=1, space="PSUM"))

        # ---------------- prep: weights + constants ----------------
        # kv-path tensors first so the serial w-chain starts ASAP
        kvT = persist.tile([128, 6, 2 * C], BF16, tag="kvT")
        nc.sync.dma_start(out=kvT, in_=kvTd[:, :, :])
        yT = persist.tile([128, 6, BPC], BF16, tag="yT")
        nc.sync.dma_start(out=yT, in_=yTd[:, :, :])
        rskb = persist.tile([128, C], F32, tag="rskb")
        nc.sync.dma_start(out=rskb, in_=rskbd[:, :])
        omask = persist.tile([128, 4, HEADS], BF16, tag="omask")
        nc.sync.dma_start(out=omask, in_=omaskd[:, :, :])
        qwT = persist.tile([128, 4, C], BF16, tag="qwT")
        nc.sync.dma_start(out=qwT, in_=qwTd[:, :, :])
        w2T = persist.tile([HEADS, 4, 128], BF16, tag="w2T")
        nc.sync.dma_start(out=w2T, in_=w2Td[:, :, :])
        gcm = persist.tile([9, 2, 9], F32, tag="gcm")
        nc.sync.dma_start(out=gcm, in_=gcmd[:, :, :])
        gcols = persist.tile([128, 12], F32, tag="gcols")
        nc.sync.dma_start(out=gcols, in_=colsd[:, :])

        ident = persist.tile([128, 128], F32, tag="ident")
        from concourse.masks import make_identity

        make_identity(nc, ident)
        identb = persist.tile([128, 128], BF16, tag="identb")
        make_identity(nc, identb)
        ones_row = persist.tile([1, 128], F32, tag="onesr")
        nc.vector.memset(ones_row, 1.0)
        ones9 = persist.tile([9, 1], F32, tag="ones9")
        nc.vector.memset(ones9, 1.0)
        zero_col = persist.tile([128, 1], F32, tag="zero")
        nc.vector.memset(zero_col, 0.0)
        nc.const_aps.aps[(F32, 0.0)] = zero_col[:, :]
        eps_col = persist.tile([128, 1], F32, tag="eps")
        nc.vector.memset(eps_col, EPS)
        nc.const_aps.aps[(F32, EPS)] = eps_col[:, :]
        # gram staging: [128 n, 4 j, 9]; col 8 of each j-block stays 1.0
        gstage = persist.tile([128, 4, 9], BF16, tag="gstage")
        nc.vector.memset(gstage[:, :, 8:9], 1.0)
        # s values for both samples: [8 h, s, n]
        s_all = persist.tile([HEADS, BPC, N], BF16, tag="s_all")

        # x for sample 0 (scalar-engine queue), sample 1 (sync queue)
        xts = []
        for s in range(BPC):
            xt = xp.tile([128, 4, N], BF16, tag="xt")
            eng = nc.scalar if s == 0 else nc.sync
            eng.dma_start(out=xt, in_=xd[s])
            xts.append(xt)

        # ---------------- ky/vy for both samples ----------------
        # ky[s, o] = sum_d y[s, d] k_w[o, d]; vy likewise (both via PE)
        krows = tiny.tile([BPC, 2, C], F32, tag="krows")
        for kv in range(2):
            ps_ky = pssm.tile([BPC, C], F32, tag="sm")
            for dt_ in range(6):
                nc.tensor.matmul(
                    ps_ky, lhsT=yT[:, dt_, :], rhs=kvT[:, dt_, kv * C : (kv + 1) * C],
                    start=(dt_ == 0), stop=(dt_ == 5),
                )
            nc.vector.tensor_copy(out=krows[:, kv, :], in_=ps_ky)
        # transpose to columns: kv_cols[p, 4*ot + 2*kv + s]
        ps_kc = pssm.tile([128, 16], F32, tag="sm")
        for ot in range(4):
            for kv in range(2):
                nc.tensor.transpose(
                    ps_kc[:, 4 * ot + 2 * kv : 4 * ot + 2 * kv + 2],
                    krows[:, kv, ts(ot, 128)],
                    ident[0:BPC, 0:BPC],
                )
        kv_cols = persist.tile([128, 16], F32, tag="kvcols")
        nc.vector.tensor_copy(out=kv_cols, in_=ps_kc)

        # ---------------- k-softmax -> w, masks ----------------
        numqs = {}

        def kv_path(s):
            # E_T[j, hd] = exp(ky[j] * rs_k[hd])
            et = ep.tile([128, 4, C], BF16, tag="eq")
            for jt in range(4):
                nc.scalar.activation(
                    out=et[:, jt, :], in_=rskb, func=AF.Exp,
                    scale=kv_cols[:, 4 * jt + s : 4 * jt + s + 1],
                )
            # masks: col 0 = vy (num), col 32 = 1 (den at psum partition 32)
            kvm = tiny.tile([128, 4, 33], BF16, tag="kvm")
            nc.vector.memset(kvm, 0.0)
            nc.vector.tensor_copy(
                out=kvm[:, :, 0:1],
                in_=kv_cols.rearrange("p (a r) -> p a r", r=4)[:, :, 2 + s : 3 + s],
            )
            nc.vector.memset(kvm[:, :, 32:33], 1.0)
            ps_w = pssm.tile([33, C], F32, tag="sm")
            for jt in range(4):
                nc.tensor.matmul(
                    ps_w, lhsT=kvm[:, jt, :], rhs=et[:, jt, :],
                    start=(jt == 0), stop=(jt == 3),
                )
            dwsb = tiny.tile([1, 2, C], F32, tag="dwsb")
            nc.vector.tensor_copy(out=dwsb[:, 0, :], in_=ps_w[32:33, :])
            nc.vector.reciprocal_approx_fast(out=dwsb[:, 1, :], in_=dwsb[:, 0, :])
            w_row = tiny.tile([1, C], F32, tag="wrow")
            nc.vector.tensor_mul(w_row, ps_w[0:1, :], dwsb[:, 1, :])
            ps_wc = pssm.tile([128, 4], F32, tag="sm")
            for ht in range(4):
                nc.tensor.transpose(
                    ps_wc[:, ht : ht + 1], w_row[:, ts(ht, 128)], ident[0:1, 0:1]
                )
            w_col = tiny.tile([128, 4], F32, tag="wcol")
            nc.vector.tensor_copy(out=w_col, in_=ps_wc)
            # numq masks: cols 0:8 = omask * w (num), cols 32:40 = omask (den)
            numq = samp.tile([128, 4, 48], BF16, tag="numq")
            nc.vector.memset(numq, 0.0)
            for ht in range(4):
                nc.vector.tensor_scalar(
                    out=numq[:, ht, 0:HEADS], in0=omask[:, ht, :],
                    scalar1=w_col[:, ht : ht + 1], scalar2=None, op0=OP.mult,
                )
            nc.vector.tensor_copy(out=numq[:, :, 32:40], in_=omask)
            numqs[s] = numq

        # ---------------- per-sample pass 1 ----------------
        psgs = {}

        def emit_gram(s, g):
            ps_sT = pssm.tile([128, 4 * HEADS], BF16, tag="sm")
            for j in range(4):
                nc.tensor.transpose(
                    ps_sT[:, 8 * j : 8 * j + 8],
                    s_all[:, s, g * GSZ + 128 * j : g * GSZ + 128 * (j + 1)],
                    identb[0:HEADS, 0:HEADS],
                )
            nc.vector.tensor_copy(
                out=gstage[:, :, 0:HEADS],
                in_=ps_sT.rearrange("p (j h) -> p j h", h=HEADS),
            )
            for j in range(4):
                nc.tensor.matmul(
                    psgs[s][:, j, :], lhsT=gstage[:, j, :], rhs=gstage[:, j, :],
                    start=(g == 0), stop=(g == NG - 1), skip_group_check=True,
                )

        def pass1_group(s, g, psnd_box, extra=None):
            """One n-group of 512: q GEMM, exp, nd matmuls; every odd group
            finishes the pair (reciprocal + muls).  Gram work for groups g-3,
            g-2 is emitted first so the PE never waits on the DVE chain.
            `extra` emits interleaved work (pass2 units of the other sample)."""
            if g >= 3 and g % 2 == 1:
                emit_gram(s, g - 3)
                emit_gram(s, g - 2)
            eq = ep.tile([128, 4, C], BF16, tag="eq")
            for ht in range(4):
                psq = psqp.tile([128, GSZ], F32, tag="psq")
                for ct in range(4):
                    nc.tensor.matmul(
                        psq,
                        lhsT=qwT[:, ct, ts(ht, 128)],
                        rhs=xts[s][:, ct, ts(g, GSZ)],
                        start=(ct == 0), stop=(ct == 3),
                    )
                nc.scalar.activation(out=eq[:, ht, :], in_=psq, func=AF.Exp)
            if g % 2 == 0:
                psnd_box[0] = psndp.tile([112, GSZ], F32, tag="nd", name="psnd")
            psnd = psnd_box[0]
            base = 64 * (g % 2)
            for ht in range(4):
                nc.tensor.matmul(
                    psnd[base : base + 48, :],
                    lhsT=numqs[s][:, ht, :], rhs=eq[:, ht, :],
                    start=(ht == 0), stop=(ht == 3),
                )
            if extra is not None:
                extra()
            if g % 2 == 1:
                dsb = gp.tile([112, GSZ], F32, tag="dsb")
                nc.vector.tensor_copy(out=dsb, in_=psnd)
                rdf = gp.tile([112, GSZ], F32, tag="rden")
                nc.vector.reciprocal_approx_fast(out=rdf, in_=dsb)
                nc.vector.tensor_mul(
                    s_all[:, s, ts(g - 1, GSZ)], psnd[0:HEADS, :], rdf[32:40, :]
                )
                nc.vector.tensor_mul(
                    s_all[:, s, ts(g, GSZ)], psnd[64 : 64 + HEADS, :],
                    rdf[96:104, :],
                )

        def stats(s):
            """GN stats from the Gram psum; DVE-only (bit-trick rsqrt) so the
            ACT exp table never swaps out."""
            psg = psgs[s]
            gsb = tiny.tile([9, 4, 9], F32, tag="gsb")
            nc.vector.tensor_copy(out=gsb, in_=psg)
            s2 = tiny.tile([9, 9], F32, tag="s2")
            nc.vector.reduce_sum(
                out=s2, in_=gsb.rearrange("p j b -> p b j"), axis=AX
            )
            work = tiny.tile([9, 2, 9], F32, tag="work")
            nc.vector.tensor_mul(work[:, 0, :], gcm[:, 0, :], s2)
            nc.vector.tensor_mul(work[:, 1, :], gcm[:, 1, :], s2)
            wred = tiny.tile([9, 2], F32, tag="wred")
            nc.vector.reduce_sum(out=wred, in_=work, axis=AX)
            ps_s = pssm.tile([1, 2], F32, tag="sm")
            nc.tensor.matmul(ps_s, lhsT=ones9, rhs=wred, start=True, stop=True)
            msc = tiny.tile([1, 4], F32, tag="msc")
            nc.vector.tensor_scalar(
                out=msc[:, 0:2], in0=ps_s, scalar1=1.0 / CN, scalar2=None,
                op0=OP.mult,
            )
            nc.vector.tensor_mul(msc[:, 2:3], msc[:, 0:1], msc[:, 0:1])
            nc.vector.tensor_sub(msc[:, 3:4], msc[:, 1:2], msc[:, 2:3])
            # rstd = rsqrt(var + eps): quake seed + 3 Newton steps, all DVE
            nt = tiny.tile([1, 12], F32, tag="nt")
            nc.vector.tensor_scalar(
                out=nt[:, 0:1], in0=msc[:, 3:4], scalar1=EPS, scalar2=None,
                op0=OP.add,
            )
            v = nt[:, 0:1]
            nt_i = nt.bitcast(mybir.dt.int32)
            nc.vector.tensor_scalar(
                out=nt_i[:, 1:2], in0=nt_i[:, 0:1], scalar1=1, scalar2=None,
                op0=OP.arith_shift_right,
            )
            nc.vector.tensor_scalar(
                out=nt_i[:, 2:3], in0=nt_i[:, 1:2], scalar1=-1,
                scalar2=0x5F3759DF, op0=OP.mult, op1=OP.add,
            )
            y = nt[:, 2:3]
            for it in range(3):
                b0 = 3 + 3 * it
                nc.vector.tensor_mul(nt[:, b0 : b0 + 1], y, y)
                nc.vector.tensor_mul(nt[:, b0 + 1 : b0 + 2], nt[:, b0 : b0 + 1], v)
                nc.vector.tensor_scalar(
                    out=nt[:, b0 + 2 : b0 + 3], in0=nt[:, b0 + 1 : b0 + 2],
                    scalar1=-0.5, scalar2=1.5, op0=OP.mult, op1=OP.add,
                )
                ynew = tiny.tile([1, 1], F32, tag=f"yn{it}")
                nc.vector.tensor_mul(ynew, y, nt[:, b0 + 2 : b0 + 3])
                y = ynew
            murow = tiny.tile([1, 2], F32, tag="murow")
            nc.vector.tensor_copy(out=murow[:, 0:1], in_=msc[:, 0:1])
            nc.vector.tensor_copy(out=murow[:, 1:2], in_=y)
            ps_b = pssm.tile([128, 2], F32, tag="sm")
            nc.tensor.matmul(ps_b, lhsT=ones_row, rhs=murow, start=True, stop=True)
            msb = tiny.tile([128, 2], F32, tag="msb")
            nc.vector.tensor_copy(out=msb, in_=ps_b)
            # A = gn_g * rstd ; B = A*(out_b - mu) + gn_b
            ab = samp.tile([128, 2, 4], F32, tag="ab")
            nc.vector.tensor_scalar(
                out=ab[:, 0, :], in0=gcols[:, 0:4],
                scalar1=msb[:, 1:2], scalar2=None, op0=OP.mult,
            )
            t1 = tiny.tile([128, 2, 4], F32, tag="t1")
            nc.vector.tensor_scalar(
                out=t1[:, 0, :], in0=gcols[:, 8:12],
                scalar1=msb[:, 0:1], scalar2=None, op0=OP.subtract,
            )
            nc.vector.tensor_mul(t1[:, 1, :], ab[:, 0, :], t1[:, 0, :])
            nc.vector.tensor_add(ab[:, 1, :], t1[:, 1, :], gcols[:, 4:8])
            return ab

        def pass2_unit(s, g, ot, idx, ab, use_psq_ring=False):
            if use_psq_ring and idx % 2 == 1:
                psf = psqp.tile([128, GSZ], F32, tag="psq", name="psf_q")
            else:
                psf = psf2p.tile([128, GSZ], F32, tag="psf")
            nc.tensor.matmul(
                psf, lhsT=w2T[:, ot, :], rhs=s_all[:, s, ts(g, GSZ)],
                start=True, stop=True,
            )
            stg = stgp.tile([128, GSZ], BF16, tag="stg")
            if idx % 3 < 2:
                nc.vector.tensor_scalar(
                    out=stg, in0=psf,
                    scalar1=ab[:, 0, ot : ot + 1], scalar2=ab[:, 1, ot : ot + 1],
                    op0=OP.mult, op1=OP.add,
                )
            else:
                nc.scalar.activation(
                    out=stg, in_=psf, func=AF.Identity,
                    scale=ab[:, 0, ot : ot + 1], bias=ab[:, 1, ot : ot + 1],
                )
            nc.sync.dma_start(out=outd[s, ot, :, ts(g, GSZ)], in_=stg)

        # pass 1 of sample 0 (sample 1's kv chain emitted at group 1 so it
        # overlaps instead of delaying the first q matmuls)
        kv_path(0)
        psgs[0] = psgp.tile([9, 4, 9], F32, tag="gram", name="psg0")
        box = [None]
        for g in range(NG):
            extra = (lambda: kv_path(1)) if g == 1 else None
            pass1_group(0, g, box, extra=extra)
        emit_gram(0, NG - 2)
        emit_gram(0, NG - 1)
        ab0 = stats(0)

        # pass 1 of sample 1, with sample 0's pass 2 interleaved
        psgs[1] = psgp.tile([9, 4, 9], F32, tag="gram", name="psg1")
        box = [None]
        for g in range(NG):
            def extra(g=g):
                for ot in range(4):
                    pass2_unit(0, g, ot, g * 4 + ot, ab0)
            pass1_group(1, g, box, extra=extra)
        emit_gram(1, NG - 2)
        emit_gram(1, NG - 1)
        ab1 = stats(1)

        # pass 2 of sample 1 (psum tiles alternate two rings: 4 in flight)
        for g in range(NG):
            for ot in range(4):
                pass2_unit(1, g, ot, g * 4 + ot, ab1, use_psq_ring=True)

    nc.finalize()
    return nc


_NC_CACHE = {}


def _get_nc():
    if "nc" not in _NC_CACHE:
        _NC_CACHE["nc"] = build_nc()
    return _NC_CACHE["nc"]


def _fold_host(inputs):
    """Host-side weight folding + staging (shared across cores)."""
    k_w = np.asarray(inputs["k_w"], np.float32)
    v_w = np.asarray(inputs["v_w"], np.float32)
    to_q_w = np.asarray(inputs["to_q_w"], np.float32)
    to_k_w = np.asarray(inputs["to_k_w"], np.float32)
    to_v_w = np.asarray(inputs["to_v_w"], np.float32)
    out_w = np.asarray(inputs["out_w"], np.float32)
    out_b = np.asarray(inputs["out_b"], np.float32)
    gn_g = np.asarray(inputs["gn_g"], np.float32)
    gn_b = np.asarray(inputs["gn_b"], np.float32)

    qwT = np.ascontiguousarray(
        to_q_w.T.reshape(4, 128, C).transpose(1, 0, 2)
    ).astype(BF)  # [128, ct, he]
    kT = k_w.T.reshape(6, 128, C).transpose(1, 0, 2)  # [128, dt, o]
    vT = v_w.T.reshape(6, 128, C).transpose(1, 0, 2)
    kvT = np.ascontiguousarray(np.concatenate([kT, vT], axis=2)).astype(BF)

    rs_k = to_k_w.sum(axis=1)  # [C]
    rs_v = to_v_w.sum(axis=1)
    rskb = np.ascontiguousarray(np.broadcast_to(rs_k[None, :], (128, C))).astype(
        np.float32
    )

    # W2[o, h] = scale * sum_e out_w[o, h*64+e] * rs_v[h*64+e]
    W2 = SCALE * np.einsum(
        "ohe,he->oh", out_w.reshape(C, HEADS, DHEAD), rs_v.reshape(HEADS, DHEAD)
    )  # [C, HEADS]
    w2T = np.ascontiguousarray(
        W2.reshape(4, 128, HEADS).transpose(2, 0, 1)
    ).astype(BF)  # [h, ot, p]

    # Gm/Cm: sum mm^q = sum_ab M[a,b] S2[a,b], S2 = [s;1][s;1]^T over n
    G = W2.T @ W2  # [8, 8]
    colsumW2 = W2.sum(axis=0)  # [8]
    bW2 = out_b @ W2  # [8]
    Gm = np.zeros((9, 9), np.float32)
    Gm[:8, :8] = G
    Gm[8, :8] = bW2
    Gm[:8, 8] = bW2
    Gm[8, 8] = float((out_b ** 2).sum())
    Cm = np.zeros((9, 9), np.float32)
    Cm[8, :8] = colsumW2 / 2.0
    Cm[:8, 8] = colsumW2 / 2.0
    Cm[8, 8] = float(out_b.sum())
    gcm = np.ascontiguousarray(
        np.stack([Cm, Gm], axis=1)
    ).astype(np.float32)  # [9, 2, 9]

    omask = np.zeros((128, 4, HEADS), np.float32)
    for ht in range(4):
        for p in range(128):
            omask[p, ht, 2 * ht + p // 64] = 1.0
    omask = omask.astype(BF)

    cols = np.ascontiguousarray(
        np.stack(
            [*gn_g.reshape(4, 128), *gn_b.reshape(4, 128), *out_b.reshape(4, 128)],
            axis=1,
        )
    ).astype(np.float32)  # [128, 12]
    return dict(qwT=qwT, kvT=kvT, rskb=rskb, omask=omask, w2T=w2T, gcm=gcm, cols=cols)


def make_in_maps(inputs):
    x = np.asarray(inputs["x"], np.float32).reshape(B, 4, 128, N)
    x = np.ascontiguousarray(x).astype(BF)
    y = np.asarray(inputs["y"], np.float32).reshape(B, DIMY)
    shared = _fold_host(inputs)
    in_maps = []
    for core in range(NCORES):
        s0 = core * BPC
        yc = y[s0 : s0 + BPC]  # [BPC, DIMY]
        yT = np.ascontiguousarray(
            yc.T.reshape(6, 128, BPC).transpose(1, 0, 2)
        ).astype(BF)
        m = {"x": x[s0 : s0 + BPC].transpose(0, 2, 1, 3).copy(), "yT": yT}
        m.update(shared)
        in_maps.append(m)
    return in_maps


def kernel(**inputs):
    nc = _get_nc()
    res = run_bass_kernel_spmd(nc, make_in_maps(inputs), list(range(NCORES)))
    out = np.concatenate([r["out"] for r in res.results], axis=0)  # [B, 4, 128, N] bf16
    return out.astype(np.float32).reshape(B, C, 64, 64)


if __name__ == "__main__":
    rng = np.random.default_rng(0)
    inputs = {
        "x": rng.standard_normal((B, C, 64, 64), dtype=np.float32),
        "y": rng.standard_normal((B, 1, 1, DIMY), dtype=np.float32),
        "k_w": rng.standard_normal((C, DIMY), dtype=np.float32) * 0.02,
        "v_w": rng.standard_normal((C, DIMY), dtype=np.float32) * 0.02,
        "to_q_w": rng.standard_normal((C, C), dtype=np.float32) * 0.02,
        "to_k_w": rng.standard_normal((C, C), dtype=np.float32) * 0.02,
        "to_v_w": rng.standard_normal((C, C), dtype=np.float32) * 0.02,
        "out_w": rng.standard_normal((C, C), dtype=np.float32) * 0.02,
        "out_b": np.zeros(C, np.float32),
        "gn_g": np.ones(C, np.float32),
        "gn_b": np.zeros(C, np.float32),
    }
    out = kernel(**inputs)
    print("kernel ran, out shape", out.shape, "std", out.std())


# revision 20
# speedup vs baseline: 1.8806x; 1.0155x over previous
"""Trainium2 Bass kernel for nn_CrossAttention (16x512x64x64, 8 heads x 64).

Math notes (exact algebraic restructuring of the reference):
  The reference tiles ky=[b,1,1,c] to k=[b,c,1,c] before conv1x1(to_k_w), so
  every input channel of that conv carries the same value ky[b,j].  Hence
    conv1x1(k, to_k_w)[b,o,0,j] = rowsum(to_k_w)[o] * ky[b,j]     (rank-1)
  and likewise for v with rowsum(to_v_w) and vy.  Propagating this:
    ksm[b,hd,j] = softmax_j(rs_k[hd] * ky[b,j])
    w[b,hd]     = sum_j ksm[b,hd,j] * vy[b,j]
    s[b,h,n]    = num/den,  num = sum_d w[hd] e^{q[hd,n]}, den = sum_d e^{q[hd,n]}
    mm[b,o,n]   = sum_h W2[o,h] * s[b,h,n] + out_b[o],
      with W2[o,h] = scale * sum_e out_w[o, h*64+e] * rs_v[h*64+e]
  followed by GroupNorm(1) over (C,H,W) per sample:
    out = A[o]*mmW2[o,n] + B[o],  A = gn_g*rstd, B = A*(out_b-mu)+gn_b
  GN stats come from the 9x9 Gram matrix of [s; 1] over n:
    sum mm   = sum_ab Cm[a,b] S2[a,b],   sum mm^2 = sum_ab Gm[a,b] S2[a,b]
  where S2 = [s;1][s;1]^T (accumulated on PE), Cm/Gm folded on host from
  W2 / out_b.

Device layout: q kept transposed [he, n] so the d-softmax reductions are
small PE matmuls (block-diagonal masks carrying w), not DVE reductions.
The only large compute is the q GEMM (to_q_w @ x, bf16, 2.1 GFLOP/sample).

Sharding: data-parallel over batch, 2 samples per core, 8 cores, no
collectives.  Weight folding (transposes, rowsums, W2, Gm/Cm) is done on
host; x is staged to bf16 on host.
"""

import numpy as np
import ml_dtypes

import concourse.bass as bass
import concourse.mybir as mybir
import concourse.tile as tile
from concourse import bacc
from concourse.bass import ts
from concourse.bass_utils import run_bass_kernel_spmd

B, C, N = 16, 512, 4096
DIMY = 768
HEADS, DHEAD = 8, 64
NCORES = 8
BPC = B // NCORES  # samples per core
SCALE = DHEAD ** -0.5
EPS = 1e-5
F32 = mybir.dt.float32
BF16 = mybir.dt.bfloat16
AX = mybir.AxisListType.X
AF = mybir.ActivationFunctionType
OP = mybir.AluOpType
NG = 8          # n-groups of 512 per sample
GSZ = N // NG   # 512
CN = C * N

BF = ml_dtypes.bfloat16


def build_nc():
    nc = bacc.Bacc()
    xd = nc.dram_tensor("x", [BPC, 128, 4, N], BF16, kind="ExternalInput")
    qwTd = nc.dram_tensor("qwT", [128, 4, C], BF16, kind="ExternalInput")
    kvTd = nc.dram_tensor("kvT", [128, 6, 2 * C], BF16, kind="ExternalInput")
    yTd = nc.dram_tensor("yT", [128, 6, BPC], BF16, kind="ExternalInput")
    rskbd = nc.dram_tensor("rskb", [128, C], F32, kind="ExternalInput")
    omaskd = nc.dram_tensor("omask", [128, 4, HEADS], BF16, kind="ExternalInput")
    w2Td = nc.dram_tensor("w2T", [HEADS, 4, 128], BF16, kind="ExternalInput")
    gcmd = nc.dram_tensor("gcm", [9, 2, 9], F32, kind="ExternalInput")
    colsd = nc.dram_tensor("cols", [128, 12], F32, kind="ExternalInput")
    outd = nc.dram_tensor("out", [BPC, 4, 128, N], BF16, kind="ExternalOutput")

    from contextlib import ExitStack

    with tile.TileContext(nc) as tc, ExitStack() as ctx:
        persist = ctx.enter_context(tc.tile_pool(name="persist", bufs=1))
        xp = ctx.enter_context(tc.tile_pool(name="xp", bufs=2))
        ep = ctx.enter_context(tc.tile_pool(name="ep", bufs=3))
        stgp = ctx.enter_context(tc.tile_pool(name="stgp", bufs=4))
        samp = ctx.enter_context(tc.tile_pool(name="samp", bufs=2))
        gp = ctx.enter_context(tc.tile_pool(name="gp", bufs=3))
        tiny = ctx.enter_context(tc.tile_pool(name="tiny", bufs=4))
        psqp = ctx.enter_context(tc.tile_pool(name="psqp", bufs=2, space="PSUM"))
        psndp = ctx.enter_context(tc.tile_pool(name="psndp", bufs=2, space="PSUM"))
        psf2p = ctx.enter_context(tc.tile_pool(name="psf2p", bufs=2, space="PSUM"))
        psgp = ctx.enter_context(tc.tile_pool(name="psgp", bufs=1, space="PSUM"))
        pssm = ctx.enter_context(tc.tile_pool(name="pssm", bufs=1, space="PSUM"))

        # ---------------- prep: weights + constants ----------------
        # The ky matmuls head the PE queue, so their inputs (yT + kvT) go
        # first, kvT in per-dt chunks so the first matmul releases early.
        yT = persist.tile([128, 6, BPC], BF16, tag="yT")
        nc.sync.dma_start(out=yT, in_=yTd[:, :, :])
        kvT = persist.tile([128, 6, 2 * C], BF16, tag="kvT")
        for dt_ in range(6):
            nc.sync.dma_start(out=kvT[:, dt_, :], in_=kvTd[:, dt_, :])
        qwT = persist.tile([128, 4, C], BF16, tag="qwT")
        nc.sync.dma_start(out=qwT, in_=qwTd[:, :, :])
        rskb = persist.tile([128, C], F32, tag="rskb")
        nc.sync.dma_start(out=rskb, in_=rskbd[:, :])
        omask = persist.tile([128, 4, HEADS], BF16, tag="omask")
        nc.sync.dma_start(out=omask, in_=omaskd[:, :, :])
        w2T = persist.tile([HEADS, 4, 128], BF16, tag="w2T")
        nc.sync.dma_start(out=w2T, in_=w2Td[:, :, :])
        gcm = persist.tile([9, 2, 9], F32, tag="gcm")
        nc.sync.dma_start(out=gcm, in_=gcmd[:, :, :])
        gcols = persist.tile([128, 12], F32, tag="gcols")
        nc.sync.dma_start(out=gcols, in_=colsd[:, :])

        ident = persist.tile([128, 128], F32, tag="ident")
        from concourse.masks import make_identity

        make_identity(nc, ident)
        identb = persist.tile([128, 128], BF16, tag="identb")
        make_identity(nc, identb)
        ones_row = persist.tile([1, 128], F32, tag="onesr")
        nc.vector.memset(ones_row, 1.0)
        ones9 = persist.tile([9, 1], F32, tag="ones9")
        nc.vector.memset(ones9, 1.0)
        zero_col = persist.tile([128, 1], F32, tag="zero")
        nc.vector.memset(zero_col, 0.0)
        nc.const_aps.aps[(F32, 0.0)] = zero_col[:, :]
        eps_col = persist.tile([128, 1], F32, tag="eps")
        nc.vector.memset(eps_col, EPS)
        nc.const_aps.aps[(F32, EPS)] = eps_col[:, :]
        # gram staging: [128 n, 4 j, 9]; col 8 of each j-block stays 1.0
        gstage = persist.tile([128, 4, 9], BF16, tag="gstage")
        nc.vector.memset(gstage[:, :, 8:9], 1.0)
        # s values for both samples: [8 h, s, n]
        s_all = persist.tile([HEADS, BPC, N], BF16, tag="s_all")

        # x: sample 0 split across three queues so the first q matmuls
        # release early; sample 1 on the sync queue behind the weights
        xts = []
        x0 = xp.tile([128, 4, N], BF16, tag="xt", name="x0")
        for ct, eng in ((0, nc.scalar), (1, nc.scalar), (2, nc.gpsimd), (3, nc.sync)):
            eng.dma_start(out=x0[:, ct, :], in_=xd[0][:, ct, :])
        x1 = xp.tile([128, 4, N], BF16, tag="xt", name="x1")
        nc.sync.dma_start(out=x1, in_=xd[1])
        xts = [x0, x1]

        # ---------------- ky/vy for both samples ----------------
        # ky[s, o] = sum_d y[s, d] k_w[o, d]; vy likewise (both via PE)
        krows = tiny.tile([BPC, 2, C], F32, tag="krows")
        for kv in range(2):
            ps_ky = pssm.tile([BPC, C], F32, tag="sm")
            for dt_ in range(6):
                nc.tensor.matmul(
                    ps_ky, lhsT=yT[:, dt_, :], rhs=kvT[:, dt_, kv * C : (kv + 1) * C],
                    start=(dt_ == 0), stop=(dt_ == 5),
                )
            nc.vector.tensor_copy(out=krows[:, kv, :], in_=ps_ky)
        # transpose to columns: kv_cols[p, 4*ot + 2*kv + s]
        ps_kc = pssm.tile([128, 16], F32, tag="sm")
        for ot in range(4):
            for kv in range(2):
                nc.tensor.transpose(
                    ps_kc[:, 4 * ot + 2 * kv : 4 * ot + 2 * kv + 2],
                    krows[:, kv, ts(ot, 128)],
                    ident[0:BPC, 0:BPC],
                )
        kv_cols = persist.tile([128, 16], F32, tag="kvcols")
        nc.vector.tensor_copy(out=kv_cols, in_=ps_kc)

        # ---------------- k-softmax -> w, masks ----------------
        numqs = {}

        def kv_path(s):
            # E_T[j, hd] = exp(ky[j] * rs_k[hd])
            et = ep.tile([128, 4, C], BF16, tag="eq")
            for jt in range(4):
                nc.scalar.activation(
                    out=et[:, jt, :], in_=rskb, func=AF.Exp,
                    scale=kv_cols[:, 4 * jt + s : 4 * jt + s + 1],
                )
            # masks: col 0 = vy (num), col 32 = 1 (den at psum partition 32)
            kvm = tiny.tile([128, 4, 33], BF16, tag="kvm")
            nc.vector.memset(kvm, 0.0)
            nc.vector.tensor_copy(
                out=kvm[:, :, 0:1],
                in_=kv_cols.rearrange("p (a r) -> p a r", r=4)[:, :, 2 + s : 3 + s],
            )
            nc.vector.memset(kvm[:, :, 32:33], 1.0)
            ps_w = pssm.tile([33, C], F32, tag="sm")
            for jt in range(4):
                nc.tensor.matmul(
                    ps_w, lhsT=kvm[:, jt, :], rhs=et[:, jt, :],
                    start=(jt == 0), stop=(jt == 3),
                )
            dwsb = tiny.tile([1, 2, C], F32, tag="dwsb")
            nc.vector.tensor_copy(out=dwsb[:, 0, :], in_=ps_w[32:33, :])
            nc.vector.reciprocal_approx_fast(out=dwsb[:, 1, :], in_=dwsb[:, 0, :])
            w_row = tiny.tile([1, C], F32, tag="wrow")
            nc.vector.tensor_mul(w_row, ps_w[0:1, :], dwsb[:, 1, :])
            ps_wc = pssm.tile([128, 4], F32, tag="sm")
            for ht in range(4):
                nc.tensor.transpose(
                    ps_wc[:, ht : ht + 1], w_row[:, ts(ht, 128)], ident[0:1, 0:1]
                )
            w_col = tiny.tile([128, 4], F32, tag="wcol")
            nc.vector.tensor_copy(out=w_col, in_=ps_wc)
            # numq masks: cols 0:8 = omask * w (num), cols 32:40 = omask (den)
            numq = samp.tile([128, 4, 48], BF16, tag="numq")
            nc.vector.memset(numq, 0.0)
            for ht in range(4):
                nc.vector.tensor_scalar(
                    out=numq[:, ht, 0:HEADS], in0=omask[:, ht, :],
                    scalar1=w_col[:, ht : ht + 1], scalar2=None, op0=OP.mult,
                )
            nc.vector.tensor_copy(out=numq[:, :, 32:40], in_=omask)
            numqs[s] = numq

        # ---------------- per-sample pass 1 ----------------
        psgs = {}

        def emit_gram(s, g):
            ps_sT = pssm.tile([128, 4 * HEADS], BF16, tag="sm")
            for j in range(4):
                nc.tensor.transpose(
                    ps_sT[:, 8 * j : 8 * j + 8],
                    s_all[:, s, g * GSZ + 128 * j : g * GSZ + 128 * (j + 1)],
                    identb[0:HEADS, 0:HEADS],
                )
            nc.vector.tensor_copy(
                out=gstage[:, :, 0:HEADS],
                in_=ps_sT.rearrange("p (j h) -> p j h", h=HEADS),
            )
            for j in range(4):
                nc.tensor.matmul(
                    psgs[s][:, j, :], lhsT=gstage[:, j, :], rhs=gstage[:, j, :],
                    start=(g == 0), stop=(g == NG - 1), skip_group_check=True,
                )

        def pass1_group(s, g, psnd_box, extra=None):
            """One n-group of 512: q GEMM, exp, nd matmuls; every odd group
            finishes the pair (reciprocal + muls).  Gram work for groups g-3,
            g-2 is emitted first so the PE never waits on the DVE chain.
            `extra` emits interleaved work (pass2 units of the other sample)."""
            if g >= 3 and g % 2 == 1:
                emit_gram(s, g - 3)
                emit_gram(s, g - 2)
            eq = ep.tile([128, 4, C], BF16, tag="eq")
            for ht in range(4):
                psq = psqp.tile([128, GSZ], F32, tag="psq")
                for ct in range(4):
                    nc.tensor.matmul(
                        psq,
                        lhsT=qwT[:, ct, ts(ht, 128)],
                        rhs=xts[s][:, ct, ts(g, GSZ)],
                        start=(ct == 0), stop=(ct == 3),
                    )
                nc.scalar.activation(out=eq[:, ht, :], in_=psq, func=AF.Exp)
            if g % 2 == 0:
                psnd_box[0] = psndp.tile([112, GSZ], F32, tag="nd", name="psnd")
            psnd = psnd_box[0]
            base = 64 * (g % 2)
            for ht in range(4):
                nc.tensor.matmul(
                    psnd[base : base + 48, :],
                    lhsT=numqs[s][:, ht, :], rhs=eq[:, ht, :],
                    start=(ht == 0), stop=(ht == 3),
                )
            if extra is not None:
                extra()
            if g % 2 == 1:
                dsb = gp.tile([112, GSZ], F32, tag="dsb")
                nc.vector.tensor_copy(out=dsb, in_=psnd)
                rdf = gp.tile([112, GSZ], F32, tag="rden")
                nc.vector.reciprocal_approx_fast(out=rdf, in_=dsb)
                nc.vector.tensor_mul(
                    s_all[:, s, ts(g - 1, GSZ)], psnd[0:HEADS, :], rdf[32:40, :]
                )
                nc.vector.tensor_mul(
                    s_all[:, s, ts(g, GSZ)], psnd[64 : 64 + HEADS, :],
                    rdf[96:104, :],
                )

        def stats(s):
            """GN stats from the Gram psum; DVE-only (bit-trick rsqrt) so the
            ACT exp table never swaps out."""
            psg = psgs[s]
            gsb = tiny.tile([9, 4, 9], F32, tag="gsb")
            nc.vector.tensor_copy(out=gsb, in_=psg)
            s2 = tiny.tile([9, 9], F32, tag="s2")
            nc.vector.reduce_sum(
                out=s2, in_=gsb.rearrange("p j b -> p b j"), axis=AX
            )
            work = tiny.tile([9, 2, 9], F32, tag="work")
            nc.vector.tensor_mul(work[:, 0, :], gcm[:, 0, :], s2)
            nc.vector.tensor_mul(work[:, 1, :], gcm[:, 1, :], s2)
            wred = tiny.tile([9, 2], F32, tag="wred")
            nc.vector.reduce_sum(out=wred, in_=work, axis=AX)
            ps_s = pssm.tile([1, 2], F32, tag="sm")
            nc.tensor.matmul(ps_s, lhsT=ones9, rhs=wred, start=True, stop=True)
            msc = tiny.tile([1, 4], F32, tag="msc")
            nc.vector.tensor_scalar(
                out=msc[:, 0:2], in0=ps_s, scalar1=1.0 / CN, scalar2=None,
                op0=OP.mult,
            )
            nc.vector.tensor_mul(msc[:, 2:3], msc[:, 0:1], msc[:, 0:1])
            nc.vector.tensor_sub(msc[:, 3:4], msc[:, 1:2], msc[:, 2:3])
            # rstd = rsqrt(var + eps): quake seed + 3 Newton steps, all DVE
            nt = tiny.tile([1, 12], F32, tag="nt")
            nc.vector.tensor_scalar(
                out=nt[:, 0:1], in0=msc[:, 3:4], scalar1=EPS, scalar2=None,
                op0=OP.add,
            )
            v = nt[:, 0:1]
            nt_i = nt.bitcast(mybir.dt.int32)
            nc.vector.tensor_scalar(
                out=nt_i[:, 1:2], in0=nt_i[:, 0:1], scalar1=1, scalar2=None,
                op0=OP.arith_shift_right,
            )
            nc.vector.tensor_scalar(
                out=nt_i[:, 2:3], in0=nt_i[:, 1:2], scalar1=-1,
                scalar2=0x5F3759DF, op0=OP.mult, op1=OP.add,
            )
            y = nt[:, 2:3]
            for it in range(3):
                b0 = 3 + 3 * it
                nc.vector.tensor_mul(nt[:, b0 : b0 + 1], y, y)
                nc.vector.tensor_mul(nt[:, b0 + 1 : b0 + 2], nt[:, b0 : b0 + 1], v)
                nc.vector.tensor_scalar(
                    out=nt[:, b0 + 2 : b0 + 3], in0=nt[:, b0 + 1 : b0 + 2],
                    scalar1=-0.5, scalar2=1.5, op0=OP.mult, op1=OP.add,
                )
                ynew = tiny.tile([1, 1], F32, tag=f"yn{it}")
                nc.vector.tensor_mul(ynew, y, nt[:, b0 + 2 : b0 + 3])
                y = ynew
            murow = tiny.tile([1, 2], F32, tag="murow")
            nc.vector.tensor_copy(out=murow[:, 0:1], in_=msc[:, 0:1])
            nc.vector.tensor_copy(out=murow[:, 1:2], in_=y)
            ps_b = pssm.tile([128, 2], F32, tag="sm")
            nc.tensor.matmul(ps_b, lhsT=ones_row, rhs=murow, start=True, stop=True)
            msb = tiny.tile([128, 2], F32, tag="msb")
            nc.vector.tensor_copy(out=msb, in_=ps_b)
            # A = gn_g * rstd ; B = A*(out_b - mu) + gn_b
            ab = samp.tile([128, 2, 4], F32, tag="ab")
            nc.vector.tensor_scalar(
                out=ab[:, 0, :], in0=gcols[:, 0:4],
                scalar1=msb[:, 1:2], scalar2=None, op0=OP.mult,
            )
            t1 = tiny.tile([128, 2, 4], F32, tag="t1")
            nc.vector.tensor_scalar(
                out=t1[:, 0, :], in0=gcols[:, 8:12],
                scalar1=msb[:, 0:1], scalar2=None, op0=OP.subtract,
            )
            nc.vector.tensor_mul(t1[:, 1, :], ab[:, 0, :], t1[:, 0, :])
            nc.vector.tensor_add(ab[:, 1, :], t1[:, 1, :], gcols[:, 4:8])
            return ab

        def pass2_unit(s, g, ot, idx, ab, tail=False):
            if tail and idx % 3 == 1:
                psf = psqp.tile([128, GSZ], F32, tag="psq", name="psf_q")
            elif tail and idx % 3 == 2:
                psf = psndp.tile([128, GSZ], F32, tag="nd", name="psf_n")
            else:
                psf = psf2p.tile([128, GSZ], F32, tag="psf")
            nc.tensor.matmul(
                psf, lhsT=w2T[:, ot, :], rhs=s_all[:, s, ts(g, GSZ)],
                start=True, stop=True,
            )
            stg = stgp.tile([128, GSZ], BF16, tag="stg")
            if (idx % 16 < 9) if tail else (idx % 3 < 2):
                nc.vector.tensor_scalar(
                    out=stg, in0=psf,
                    scalar1=ab[:, 0, ot : ot + 1], scalar2=ab[:, 1, ot : ot + 1],
                    op0=OP.mult, op1=OP.add,
                )
            else:
                nc.scalar.activation(
                    out=stg, in_=psf, func=AF.Identity,
                    scale=ab[:, 0, ot : ot + 1], bias=ab[:, 1, ot : ot + 1],
                )
            nc.sync.dma_start(out=outd[s, ot, :, ts(g, GSZ)], in_=stg)

        # pass 1 of sample 0 (sample 1's kv chain emitted at group 1 so it
        # overlaps instead of delaying the first q matmuls)
        kv_path(0)
        psgs[0] = psgp.tile([9, 4, 9], F32, tag="gram", name="psg0")
        box = [None]
        for g in range(NG):
            extra = (lambda: kv_path(1)) if g == 1 else None
            pass1_group(0, g, box, extra=extra)
        emit_gram(0, NG - 2)
        emit_gram(0, NG - 1)
        ab0 = stats(0)

        # pass 1 of sample 1, with sample 0's pass 2 interleaved
        psgs[1] = psgp.tile([9, 4, 9], F32, tag="gram", name="psg1")
        box = [None]
        for g in range(NG):
            def extra(g=g):
                for ot in range(4):
                    pass2_unit(0, g, ot, g * 4 + ot, ab0)
            pass1_group(1, g, box, extra=extra)
        emit_gram(1, NG - 2)
        emit_gram(1, NG - 1)
        ab1 = stats(1)

        # pass 2 of sample 1 (psum tiles alternate two rings: 4 in flight)
        for g in range(NG):
            for ot in range(4):
                pass2_unit(1, g, ot, g * 4 + ot, ab1, tail=True)

    nc.finalize()
    return nc


_NC_CACHE = {}


def _get_nc():
    if "nc" not in _NC_CACHE:
        _NC_CACHE["nc"] = build_nc()
    return _NC_CACHE["nc"]


def _fold_host(inputs):
    """Host-side weight folding + staging (shared across cores)."""
    k_w = np.asarray(inputs["k_w"], np.float32)
    v_w = np.asarray(inputs["v_w"], np.float32)
    to_q_w = np.asarray(inputs["to_q_w"], np.float32)
    to_k_w = np.asarray(inputs["to_k_w"], np.float32)
    to_v_w = np.asarray(inputs["to_v_w"], np.float32)
    out_w = np.asarray(inputs["out_w"], np.float32)
    out_b = np.asarray(inputs["out_b"], np.float32)
    gn_g = np.asarray(inputs["gn_g"], np.float32)
    gn_b = np.asarray(inputs["gn_b"], np.float32)

    qwT = np.ascontiguousarray(
        to_q_w.T.reshape(4, 128, C).transpose(1, 0, 2)
    ).astype(BF)  # [128, ct, he]
    kT = k_w.T.reshape(6, 128, C).transpose(1, 0, 2)  # [128, dt, o]
    vT = v_w.T.reshape(6, 128, C).transpose(1, 0, 2)
    kvT = np.ascontiguousarray(np.concatenate([kT, vT], axis=2)).astype(BF)

    rs_k = to_k_w.sum(axis=1)  # [C]
    rs_v = to_v_w.sum(axis=1)
    rskb = np.ascontiguousarray(np.broadcast_to(rs_k[None, :], (128, C))).astype(
        np.float32
    )

    # W2[o, h] = scale * sum_e out_w[o, h*64+e] * rs_v[h*64+e]
    W2 = SCALE * np.einsum(
        "ohe,he->oh", out_w.reshape(C, HEADS, DHEAD), rs_v.reshape(HEADS, DHEAD)
    )  # [C, HEADS]
    w2T = np.ascontiguousarray(
        W2.reshape(4, 128, HEADS).transpose(2, 0, 1)
    ).astype(BF)  # [h, ot, p]

    # Gm/Cm: sum mm^q = sum_ab M[a,b] S2[a,b], S2 = [s;1][s;1]^T over n
    G = W2.T @ W2  # [8, 8]
    colsumW2 = W2.sum(axis=0)  # [8]
    bW2 = out_b @ W2  # [8]
    Gm = np.zeros((9, 9), np.float32)
    Gm[:8, :8] = G
    Gm[8, :8] = bW2
    Gm[:8, 8] = bW2
    Gm[8, 8] = float((out_b ** 2).sum())
    Cm = np.zeros((9, 9), np.float32)
    Cm[8, :8] = colsumW2 / 2.0
    Cm[:8, 8] = colsumW2 / 2.0
    Cm[8, 8] = float(out_b.sum())
    gcm = np.ascontiguousarray(
        np.stack([Cm, Gm], axis=1)
    ).astype(np.float32)  # [9, 2, 9]

    omask = np.zeros((128, 4, HEADS), np.float32)
    for ht in range(4):
        for p in range(128):
            omask[p, ht, 2 * ht + p // 64] = 1.0
    omask = omask.astype(BF)

    cols = np.ascontiguousarray(
        np.stack(
            [*gn_g.reshape(4, 128), *gn_b.reshape(4, 128), *out_b.reshape(4, 128)],
            axis=1,
        )
    ).astype(np.float32)  # [128, 12]
    return dict(qwT=qwT, kvT=kvT, rskb=rskb, omask=omask, w2T=w2T, gcm=gcm, cols=cols)


def make_in_maps(inputs):
    x = np.asarray(inputs["x"], np.float32).reshape(B, 4, 128, N)
    x = np.ascontiguousarray(x).astype(BF)
    y = np.asarray(inputs["y"], np.float32).reshape(B, DIMY)
    shared = _fold_host(inputs)
    in_maps = []
    for core in range(NCORES):
        s0 = core * BPC
        yc = y[s0 : s0 + BPC]  # [BPC, DIMY]
        yT = np.ascontiguousarray(
            yc.T.reshape(6, 128, BPC).transpose(1, 0, 2)
        ).astype(BF)
        m = {"x": x[s0 : s0 + BPC].transpose(0, 2, 1, 3).copy(), "yT": yT}
        m.update(shared)
        in_maps.append(m)
    return in_maps


def kernel(**inputs):
    nc = _get_nc()
    res = run_bass_kernel_spmd(nc, make_in_maps(inputs), list(range(NCORES)))
    out = np.concatenate([r["out"] for r in res.results], axis=0)  # [B, 4, 128, N] bf16
    return out.astype(np.float32).reshape(B, C, 64, 64)


if __name__ == "__main__":
    rng = np.random.default_rng(0)
    inputs = {
        "x": rng.standard_normal((B, C, 64, 64), dtype=np.float32),
        "y": rng.standard_normal((B, 1, 1, DIMY), dtype=np.float32),
        "k_w": rng.standard_normal((C, DIMY), dtype=np.float32) * 0.02,
        "v_w": rng.standard_normal((C, DIMY), dtype=np.float32) * 0.02,
        "to_q_w": rng.standard_normal((C, C), dtype=np.float32) * 0.02,
        "to_k_w": rng.standard_normal((C, C), dtype=np.float32) * 0.02,
        "to_v_w": rng.standard_normal((C, C), dtype=np.float32) * 0.02,
        "out_w": rng.standard_normal((C, C), dtype=np.float32) * 0.02,
        "out_b": np.zeros(C, np.float32),
        "gn_g": np.ones(C, np.float32),
        "gn_b": np.zeros(C, np.float32),
    }
    out = kernel(**inputs)
    print("kernel ran, out shape", out.shape, "std", out.std())
